# revision 1
# baseline (speedup 1.0000x reference)
"""Trainium2 Bass kernel for nn_EnhancedFlowLayer (topk_masking).

8 cores. Tokens on partitions (2 groups of 128); flow (i,j)-space sharded by i
across cores (64 i-rows -> 32768 elems/token/core). flow is rematerialized on
the PE per phase and never hits HBM. Per-token exact rank-kk threshold via:
bf16 |F| + sampled Newton + exact 5-rung count ladder (one all-reduce) + band
extraction (top-2 per 64-chunk) + one all-gather + replicated exact fp32
bisection. Final pass recomputes F, applies mask, does the masked matvec, one
all-gather of flow_out slices, then a replicated LN2 + memory-MLP + FFN tail.
"""

import os
from contextlib import ExitStack

import numpy as np

B, S, D, P = 1, 256, 512, 16
MAX_SEQ = 4096
NCORES = 8
ISLICE = D // NCORES          # 64 i-rows per core
FREE = ISLICE * D             # 32768 ij elements per token per core
NG = 2                        # token groups of 128
DD = D * D
HF = FREE // 2                # 16384
NCH = HF // 64                # 256 chunks of 64 per half
NCAND = 6 * NCH               # candidate slots per token per core (top-3 x 2 halves)
NL = 5                        # ladder rungs
N_BISECT = 11
N_BISECT2 = 17
QF = FREE // 4            # 8192 count-scratch width

DEBUG = os.environ.get("KERNEL_DEBUG", "0") == "1"
STAGE = int(os.environ.get("KERNEL_STAGE", "3"))
MM_DT_NAME = os.environ.get("KERNEL_MM_DT", "float32")
SIM_COMPAT = os.environ.get("KERNEL_SIM_COMPAT", "0") == "1"


def _host_constants():
    pos = np.arange(S, dtype=np.float64)
    inv = 1.0 / (10000.0 ** (np.arange(0, D, 2, dtype=np.float64) / D))
    ang = pos[:, None] * inv[None, :]
    sin = np.repeat(np.sin(ang), 2, axis=-1).astype(np.float32)
    cos = np.repeat(np.cos(ang), 2, axis=-1).astype(np.float32)
    # half-normal tail quantile z(q): P(|N(0,1)| >= z) = q, cubic in ln q
    qpoly = np.array([-0.0036756, -0.06789169, -0.73664117, 0.26370117], np.float32)
    return sin, cos, qpoly


def build_kernel():
    import concourse.bass as bass
    import concourse.mybir as mybir
    from concourse import bacc, masks
    from concourse.tile import TileContext

    dt = mybir.dt
    Alu = mybir.AluOpType
    Act = mybir.ActivationFunctionType
    AxX = mybir.AxisListType.X
    f32, bf16 = dt.float32, dt.bfloat16
    MM_DT = getattr(dt, MM_DT_NAME)

    nc = bacc.Bacc("TRN2", num_devices=NCORES)

    def mmc(ap):
        return ap.bitcast(MM_DT) if MM_DT != f32 else ap

    dp = nc.declare_dram_parameter
    x_in = dp("x", [S, D], f32, isOutput=False)
    pat_sl = dp("pat_sl", [P, FREE], f32, isOutput=False)
    sel_w1 = dp("sel_w1", [2 * D, 2 * P], f32, isOutput=False)
    sel_b1 = dp("sel_b1", [1, 2 * P], f32, isOutput=False)
    sel_w2 = dp("sel_w2", [2 * P, P], f32, isOutput=False)
    sel_b2 = dp("sel_b2", [1, P], f32, isOutput=False)
    win_w1 = dp("win_w1", [D, 64], f32, isOutput=False)
    win_b1 = dp("win_b1", [1, 64], f32, isOutput=False)
    win_w2 = dp("win_w2", [64, 1], f32, isOutput=False)
    win_b2 = dp("win_b2", [1, 1], f32, isOutput=False)
    int_w1 = dp("int_w1", [2 * D, 64], f32, isOutput=False)
    int_b1 = dp("int_b1", [1, 64], f32, isOutput=False)
    int_w2 = dp("int_w2", [64, 1], f32, isOutput=False)
    int_b2 = dp("int_b2", [1, 1], f32, isOutput=False)
    mem_w1 = dp("mem_w1", [2 * D, D], f32, isOutput=False)
    mem_b1 = dp("mem_b1", [1, D], f32, isOutput=False)
    mem_w2 = dp("mem_w2", [D, D], f32, isOutput=False)
    mem_b2 = dp("mem_b2", [1, D], f32, isOutput=False)
    memory_bank = dp("memory_bank", [512, D], f32, isOutput=False)
    up_w = dp("up_w", [D, 8 * D], f32, isOutput=False)
    up_b = dp("up_b", [1, 8 * D], f32, isOutput=False)
    down_w = dp("down_w", [4 * D, D], f32, isOutput=False)
    down_b = dp("down_b", [1, D], f32, isOutput=False)
    n1_g = dp("n1_g", [1, D], f32, isOutput=False)
    n1_b = dp("n1_b", [1, D], f32, isOutput=False)
    n2_g = dp("n2_g", [1, D], f32, isOutput=False)
    n2_b = dp("n2_b", [1, D], f32, isOutput=False)
    rope_sin = dp("rope_sin", [S, D], f32, isOutput=False)
    rope_cos = dp("rope_cos", [S, D], f32, isOutput=False)
    qpoly = dp("qpoly", [1, 4], f32, isOutput=False)
    out_dram = dp("out", [S, D], f32, isOutput=True)

    dbg = {}
    if DEBUG:
        for name, shape in [
            ("dbg_xn", [S, D]), ("dbg_xr", [S, D]), ("dbg_pw", [S, P]),
            ("dbg_inten", [S, 1]), ("dbg_scal", [1, 8]), ("dbg_t0", [S, 1]),
            ("dbg_cnt", [S, 8]), ("dbg_beta", [S, 4]), ("dbg_th", [S, 2]),
            ("dbg_fo", [S, D]), ("dbg_cand", [S, NCAND]),
        ]:
            dbg[name] = dp(name, shape, f32, isOutput=True)

    RG = [list(range(NCORES))]

    with ExitStack() as ctx:
        tc = ctx.enter_context(TileContext(nc))
        # persistent small state (lives for whole kernel)
        pw_ = ctx.enter_context(tc.tile_pool(name="persist", bufs=1))
        # PSUM pools: 6 banks matmul + 2 banks transposes/misc
        pool_mm = ctx.enter_context(tc.tile_pool(name="psumMM", bufs=6, space="PSUM"))
        pool_ps = ctx.enter_context(tc.tile_pool(name="psumT", bufs=2, space="PSUM"))
        pool_dram = ctx.enter_context(tc.tile_pool(name="dramst", bufs=1, space="DRAM"))

        def dma(dst, src):
            nc.sync.dma_start(out=dst, in_=src)

        def bcast_row(pool, src_dram_row, width, name, dtype=f32):
            t = pool.tile([128, width], dtype, name=name)
            dma(t[:], src_dram_row[:].to_broadcast([128, width]))
            return t

        identity = pw_.tile([128, 128], f32, name="identity")
        masks.make_identity(nc, identity[:])
        bc_n = [0]

        def pbcast(pool, dst_ap, src_ap, width, name):
            """broadcast [1,width] sbuf row to [128,width] via a DRAM bounce"""
            bc_n[0] += 1
            st = pool_dram.tile([1, width], f32, name=f"bc{bc_n[0]}_{name}")
            dma(st[:], src_ap)
            dma(dst_ap, st[:].to_broadcast([128, width]))

        def transpose_to(dst_ap, src_ap, name):
            p, f = src_ap.shape[0], src_ap.free_size()
            ps = pool_ps.tile([f, p], f32, name="Tps", tag="Tps",
                              padded_shape=[128, 128])
            nc.tensor.transpose(ps[:f, :p], src_ap, identity[:p, :p])
            nc.vector.tensor_copy(dst_ap, ps[:f, :p])

        ERF_FN = Act.Tanh if SIM_COMPAT else Act.Erf

        def gelu_(pool, ap, name):
            """in-place exact gelu: x * 0.5*(1+erf(x/sqrt(2)))"""
            e = pool.tile(list(ap.shape), f32, name=f"{name}_erf", tag="gelu_e")
            nc.scalar.activation(e[:], ap, ERF_FN, scale=float(1 / np.sqrt(2)))
            nc.vector.tensor_scalar(e[:], e[:], 1.0, 0.5, Alu.add, Alu.mult)
            nc.vector.tensor_tensor(ap, ap, e[:], Alu.mult)

        def silu_(pool, dst_ap, src_ap, name):
            """dst = src * sigmoid(src) (exact identity)"""
            sg = pool.tile(list(src_ap.shape), f32, name=f"{name}_sg", tag="silu_s")
            nc.scalar.activation(sg[:], src_ap, Act.Sigmoid)
            nc.vector.tensor_tensor(dst_ap, src_ap, sg[:], Alu.mult)

        # ---------- persistent tiles ----------
        xg = [pw_.tile([128, D], f32, name=f"xg{g}") for g in range(NG)]
        xn = [pw_.tile([128, D], f32, name=f"xn{g}") for g in range(NG)]
        pwt = [pw_.tile([P, 128], f32, name=f"pwT{g}") for g in range(NG)]
        inten = [pw_.tile([128, 1], f32, name=f"inten{g}") for g in range(NG)]
        kk_b = pw_.tile([128, 1], f32, name="kk_b")
        zq_b = pw_.tile([128, 1], f32, name="zq_b")
        delta_b = pw_.tile([128, 1], f32, name="delta_b")
        invz2_b = pw_.tile([128, 1], f32, name="invz2_b")
        ones_sb = pw_.tile([128, 1], f32, name="ones_sb")
        nc.vector.memset(ones_sb[:], 1.0)
        beta = [(pw_.tile([128, 1], f32, name=f"b1t{g}"),
                 pw_.tile([128, 1], f32, name=f"b2t{g}")) for g in range(NG)]
        rprime = [pw_.tile([128, 1], f32, name=f"rp{g}") for g in range(NG)]
        th = [pw_.tile([128, 1], f32, name=f"th{g}") for g in range(NG)]

        for g in range(NG):
            dma(xg[g][:], x_in[g * 128:(g + 1) * 128, :])

        # =================== preamble (scoped pool) ===================
        with tc.tile_pool(name="preamble", bufs=1) as pp:
            sin_g, cos_g, xr = [], [], []
            for g in range(NG):
                t = pp.tile([128, D], f32, name=f"sin{g}")
                dma(t[:], rope_sin[g * 128:(g + 1) * 128, :])
                sin_g.append(t)
                t = pp.tile([128, D], f32, name=f"cos{g}")
                dma(t[:], rope_cos[g * 128:(g + 1) * 128, :])
                cos_g.append(t)
            n1g_b = bcast_row(pp, n1_g, D, "n1g_b")
            n1b_b = bcast_row(pp, n1_b, D, "n1b_b")

            for g in range(NG):
                mean = pp.tile([128, 1], f32, name=f"mean{g}")
                m2 = pp.tile([128, 1], f32, name=f"m2ln{g}")
                tmp = pp.tile([128, D], f32, name=f"lntmp{g}")
                nc.vector.tensor_reduce(mean[:], xg[g][:], AxX, Alu.add)
                nc.vector.tensor_scalar(mean[:], mean[:], 1.0 / D, None, Alu.mult)
                nc.vector.tensor_scalar(tmp[:], xg[g][:], mean[:], None, Alu.subtract)
                nc.vector.scalar_tensor_tensor(tmp[:], tmp[:], 1.0, tmp[:], Alu.mult,
                                               Alu.mult, accum_out=m2[:])
                nc.vector.tensor_scalar(m2[:], m2[:], 1.0 / D, 1e-5, Alu.mult, Alu.add)
                rstd = pp.tile([128, 1], f32, name=f"rstd{g}")
                nc.scalar.activation(rstd[:], m2[:], Act.Sqrt)
                nc.vector.reciprocal(rstd[:], rstd[:])
                nc.vector.tensor_scalar(xn[g][:], xg[g][:], mean[:], rstd[:],
                                        Alu.subtract, Alu.mult)
                nc.vector.scalar_tensor_tensor(xn[g][:], xn[g][:], 1.0, n1g_b[:],
                                               Alu.mult, Alu.mult)
                nc.vector.tensor_tensor(xn[g][:], xn[g][:], n1b_b[:], Alu.add)
                t_xr = pp.tile([128, D], f32, name=f"xr{g}")
                rot = pp.tile([128, D], f32, name=f"rot{g}")
                ev = lambda a: a.rearrange("p (a two) -> p a two", two=2)[:, :, 0]
                od = lambda a: a.rearrange("p (a two) -> p a two", two=2)[:, :, 1]
                nc.vector.tensor_scalar(ev(rot[:]), od(xn[g][:]), -1.0, None, Alu.mult)
                nc.vector.tensor_copy(od(rot[:]), ev(xn[g][:]))
                nc.vector.tensor_tensor(rot[:], rot[:], sin_g[g][:], Alu.mult)
                nc.vector.scalar_tensor_tensor(t_xr[:], xn[g][:], 1.0, cos_g[g][:],
                                               Alu.mult, Alu.mult)
                nc.vector.tensor_tensor(t_xr[:], t_xr[:], rot[:], Alu.add)
                xr.append(t_xr)

            # ctx = mean over tokens
            ctx_ps = pool_ps.tile([1, D], f32, name="ctx_ps", tag="Tps",
                                  padded_shape=[128, 512])
            for g in range(NG):
                nc.tensor.matmul(ctx_ps[:1, :], ones_sb[:], xr[g][:],
                                 start=(g == 0), stop=(g == NG - 1))
            ctx_row = pp.tile([1, D], f32, name="ctx_row")
            nc.vector.tensor_scalar(ctx_row[:], ctx_ps[:1, :], 1.0 / S, None, Alu.mult)

            xrT = pp.tile([128, 4 * S], f32, name="xrT")
            for g in range(NG):
                for kc in range(4):
                    transpose_to(xrT[:, kc * S + g * 128: kc * S + (g + 1) * 128],
                                 xr[g][:, kc * 128:(kc + 1) * 128], f"xrT{g}{kc}")
            ctxT = pp.tile([128, 4], f32, name="ctxT")
            for kc in range(4):
                transpose_to(ctxT[:, kc:kc + 1], ctx_row[:, kc * 128:(kc + 1) * 128],
                             f"ctxT{kc}")

            def mlp_head(w1, b1, w2, b2, h1_dim, h2_dim, name):
                w1a = pp.tile([128, 4 * h1_dim], f32, name=f"{name}_w1a")
                w1b = pp.tile([128, 4 * h1_dim], f32, name=f"{name}_w1b")
                for kc in range(4):
                    dma(w1a[:, kc * h1_dim:(kc + 1) * h1_dim],
                        w1[kc * 128:(kc + 1) * 128, :])
                    dma(w1b[:, kc * h1_dim:(kc + 1) * h1_dim],
                        w1[D + kc * 128: D + (kc + 1) * 128, :])
                b1_b = bcast_row(pp, b1, h1_dim, f"{name}_b1b")
                w2_sb = pp.tile([h1_dim, h2_dim], f32, name=f"{name}_w2sb")
                dma(w2_sb[:], w2[:])
                b2_b = bcast_row(pp, b2, h2_dim, f"{name}_b2b")
                v1_ps = pool_ps.tile([1, h1_dim], f32, name="v1ps", tag="Tps",
                                     padded_shape=[128, 128])
                for kc in range(4):
                    nc.tensor.matmul(v1_ps[:1, :], ctxT[:, kc:kc + 1],
                                     w1b[:, kc * h1_dim:(kc + 1) * h1_dim],
                                     start=(kc == 0), stop=(kc == 3))
                v1 = pp.tile([1, h1_dim], f32, name=f"{name}_v1")
                nc.vector.tensor_copy(v1[:], v1_ps[:1, :])
                v1_b = pp.tile([128, h1_dim], f32, name=f"{name}_v1b")
                pbcast(pp, v1_b[:], v1[:], h1_dim, f"{name}v1")
                outs = []
                for g in range(NG):
                    h1_ps = pool_ps.tile([128, h1_dim], f32, name="h1ps", tag="Tps",
                                         padded_shape=[128, 128])
                    for kc in range(4):
                        nc.tensor.matmul(
                            h1_ps[:], xrT[:, kc * S + g * 128: kc * S + (g + 1) * 128],
                            w1a[:, kc * h1_dim:(kc + 1) * h1_dim],
                            start=(kc == 0), stop=(kc == 3))
                    h1 = pp.tile([128, h1_dim], f32, name=f"{name}_h1_{g}")
                    nc.vector.tensor_tensor(h1[:], h1_ps[:], v1_b[:], Alu.add)
                    nc.vector.tensor_tensor(h1[:], h1[:], b1_b[:], Alu.add)
                    gelu_(pp, h1[:], f"{name}g{g}")
                    h1T = pp.tile([h1_dim, 128], f32, name=f"{name}_h1T_{g}")
                    transpose_to(h1T[:], h1[:], f"{name}h1T{g}")
                    h2_ps = pool_ps.tile([128, h2_dim], f32, name="h2ps", tag="Tps",
                                         padded_shape=[128, 128])
                    nc.tensor.matmul(h2_ps[:], h1T[:], w2_sb[:], start=True, stop=True)
                    h2 = pp.tile([128, h2_dim], f32, name=f"{name}_h2_{g}")
                    nc.vector.tensor_tensor(h2[:], h2_ps[:], b2_b[:], Alu.add)
                    outs.append(h2)
                return outs

            sel_h2 = mlp_head(sel_w1, sel_b1, sel_w2, sel_b2, 2 * P, P, "sel")
            int_h2 = mlp_head(int_w1, int_b1, int_w2, int_b2, 64, 1, "intm")

            for g in range(NG):
                t_pw = pp.tile([128, P], f32, name=f"pwsm{g}")
                mx = pp.tile([128, 1], f32, name=f"selmx{g}")
                nc.vector.tensor_reduce(mx[:], sel_h2[g][:], AxX, Alu.max)
                nc.vector.tensor_scalar(sel_h2[g][:], sel_h2[g][:], mx[:], None,
                                        Alu.subtract)
                nc.scalar.activation(sel_h2[g][:], sel_h2[g][:], Act.Exp)
                sm = pp.tile([128, 1], f32, name=f"selsm{g}")
                nc.vector.tensor_reduce(sm[:], sel_h2[g][:], AxX, Alu.add)
                rs = pp.tile([128, 1], f32, name=f"selrs{g}")
                nc.vector.reciprocal(rs[:], sm[:])
                nc.vector.tensor_scalar(t_pw[:], sel_h2[g][:], rs[:], None, Alu.mult)
                nc.scalar.activation(inten[g][:], int_h2[g][:], Act.Sigmoid)
                transpose_to(pwt[g][:], t_pw[:], f"pwT{g}")
                if DEBUG:
                    dma(dbg["dbg_pw"][g * 128:(g + 1) * 128, :], t_pw[:])

            # window scalar -> kk, z, delta
            winw1_sb = pp.tile([128, 4 * 64], f32, name="winw1_sb")
            for kc in range(4):
                dma(winw1_sb[:, kc * 64:(kc + 1) * 64],
                    win_w1[kc * 128:(kc + 1) * 128, :])
            wh1_ps = pool_ps.tile([1, 64], f32, name="wh1ps", tag="Tps",
                                  padded_shape=[128, 128])
            for kc in range(4):
                nc.tensor.matmul(wh1_ps[:1, :], ctxT[:, kc:kc + 1],
                                 winw1_sb[:, kc * 64:(kc + 1) * 64],
                                 start=(kc == 0), stop=(kc == 3))
            wh1 = pp.tile([1, 64], f32, name="wh1")
            wb1_sb = pp.tile([1, 64], f32, name="wb1_sb")
            dma(wb1_sb[:], win_b1[:])
            nc.vector.tensor_tensor(wh1[:], wh1_ps[:1, :], wb1_sb[:], Alu.add)
            gelu_(pp, wh1[:], "wh1g")
            wh1T = pp.tile([64, 1], f32, name="wh1T")
            transpose_to(wh1T[:], wh1[:], "wh1T")
            winw2_sb = pp.tile([64, 1], f32, name="winw2_sb")
            dma(winw2_sb[:], win_w2[:])
            win_ps = pool_ps.tile([1, 1], f32, name="winps", tag="Tps",
                                  padded_shape=[128, 128])
            nc.tensor.matmul(win_ps[:1, :1], wh1T[:], winw2_sb[:], start=True,
                             stop=True)
            winv = pp.tile([1, 1], f32, name="winv")
            wb2_sb = pp.tile([1, 1], f32, name="wb2_sb")
            dma(wb2_sb[:], win_b2[:])
            nc.vector.tensor_tensor(winv[:], win_ps[:1, :1], wb2_sb[:], Alu.add)
            nc.scalar.activation(winv[:], winv[:], Act.Sigmoid)
            nc.vector.tensor_scalar(winv[:], winv[:], float(MAX_SEQ - 256), 256.0,
                                    Alu.mult, Alu.add)
            kkf = pp.tile([1, 1], f32, name="kkf")
            nc.vector.tensor_scalar(kkf[:], winv[:], 0.1 / MAX_SEQ * DD, None,
                                    Alu.mult)
            # floor() robust to the f32->i32 convert rounding mode
            ki = pp.tile([1, 1], dt.int32, name="ki")
            nc.vector.tensor_copy(ki[:], kkf[:])
            kf2 = pp.tile([1, 1], f32, name="kf2")
            nc.vector.tensor_copy(kf2[:], ki[:])
            kgt = pp.tile([1, 1], f32, name="kgt")
            nc.vector.tensor_tensor(kgt[:], kf2[:], kkf[:], Alu.is_gt)
            nc.vector.tensor_tensor(kkf[:], kf2[:], kgt[:], Alu.subtract)
            nc.vector.tensor_scalar(kkf[:], kkf[:], 1.0, None, Alu.max)

            qp = pp.tile([1, 4], f32, name="qp")
            dma(qp[:], qpoly[:])
            u = pp.tile([1, 1], f32, name="qu")
            nc.vector.tensor_scalar(u[:], kkf[:], 1.0 / DD, None, Alu.mult)
            nc.scalar.activation(u[:], u[:], Act.Ln)
            zq = pp.tile([1, 1], f32, name="zq")
            nc.vector.tensor_scalar(zq[:], qp[:, 0:1], u[:], qp[:, 1:2], Alu.mult,
                                    Alu.add)
            nc.vector.tensor_scalar(zq[:], zq[:], u[:], qp[:, 2:3], Alu.mult, Alu.add)
            nc.vector.tensor_scalar(zq[:], zq[:], u[:], qp[:, 3:4], Alu.mult, Alu.add)
            phi = pp.tile([1, 1], f32, name="phi")
            nc.vector.scalar_tensor_tensor(phi[:], zq[:], -0.5, zq[:], Alu.mult,
                                           Alu.mult)
            nc.scalar.activation(phi[:], phi[:], Act.Exp)
            nc.vector.tensor_scalar(phi[:], phi[:], float(1.0 / np.sqrt(2 * np.pi)),
                                    None, Alu.mult)
            dens = pp.tile([1, 1], f32, name="dens")
            nc.vector.scalar_tensor_tensor(dens[:], phi[:], float(2.0 * DD), zq[:],
                                           Alu.mult, Alu.mult)
            delta = pp.tile([1, 1], f32, name="delta")
            nc.vector.reciprocal(delta[:], dens[:])
            nc.vector.tensor_scalar(delta[:], delta[:], 700.0, None, Alu.mult)
            pbcast(pp, kk_b[:], kkf[:], 1, "kk")
            pbcast(pp, zq_b[:], zq[:], 1, "zq")
            pbcast(pp, delta_b[:], delta[:], 1, "delta")
            nc.vector.scalar_tensor_tensor(invz2_b[:], zq_b[:], 1.0, zq_b[:],
                                           Alu.mult, Alu.mult)
            nc.vector.reciprocal(invz2_b[:], invz2_b[:])

            if DEBUG:
                for g in range(NG):
                    dma(dbg["dbg_xn"][g * 128:(g + 1) * 128, :], xn[g][:])
                    dma(dbg["dbg_xr"][g * 128:(g + 1) * 128, :], xr[g][:])
                    dma(dbg["dbg_inten"][g * 128:(g + 1) * 128, :], inten[g][:])
                dma(dbg["dbg_scal"][:, 0:1], kkf[:])
                dma(dbg["dbg_scal"][:, 1:2], winv[:])
                dma(dbg["dbg_scal"][:, 2:3], zq[:])
                dma(dbg["dbg_scal"][:, 3:4], delta[:])

        if STAGE < 2:
            for g in range(NG):
                dma(out_dram[g * 128:(g + 1) * 128, :], xg[g][:])
            return nc

        # =========== helper: stream patterns & rematerialize F ===========
        def flow_pass(g, consume, pat_pool, wlist=None):
            """consume(c, psum_ap) for each 512-chunk c (i_loc = c) of group g."""
            for w in (wlist if wlist is not None else range(16)):
                patw = pat_pool.tile([P, 2048], f32, name="patw", tag="patw", bufs=3)
                dma(patw[:], pat_sl[:, w * 2048:(w + 1) * 2048])
                for m in range(4):
                    c = w * 4 + m
                    ps = pool_mm.tile([128, 512], f32, name="Fps", tag="Fps")
                    nc.tensor.matmul(ps[:], mmc(pwt[g][:]),
                                     mmc(patw[:, m * 512:(m + 1) * 512]),
                                     start=True, stop=True)
                    consume(c, ps)

        t0_stage = pool_dram.tile([S, 1], f32, name="t0_stage")
        t0_out = pool_dram.tile([S, 1], f32, name="t0_out", addr_space="Shared")
        cnt_stage = pool_dram.tile([S, NL], f32, name="cnt_stage")
        cnt_out = pool_dram.tile([S, NL], f32, name="cnt_out", addr_space="Shared")
        cand_stage = pool_dram.tile([S, NCAND], f32, name="cand_stage")
        cand_out = pool_dram.tile([NCORES, S, NCAND], f32, name="cand_out",
                                  addr_space="Shared")

        tlad_all = []
        # =============== P1 + selection ladder (scoped pool) ===============
        with tc.tile_pool(name="selpool", bufs=1) as sp:
            A_bf = sp.tile([128, NG * FREE], bf16, name="A_bf")
            scratch = sp.tile([128, QF], bf16, name="scratch")

            for g in range(NG):
                def consume_p1(c, ps, g=g):
                    nc.scalar.activation(
                        A_bf[:, g * FREE + c * 512: g * FREE + (c + 1) * 512],
                        ps[:], Act.Abs, scale=inten[g][:])
                flow_pass(g, consume_p1, sp)

            for g in range(NG):
                Ag = A_bf[:, g * FREE:(g + 1) * FREE]
                m4 = sp.tile([128, 4], f32, name=f"m4_{g}")
                for q in range(4):
                    nc.vector.scalar_tensor_tensor(
                        scratch[:], Ag[:, q * QF:(q + 1) * QF], 1.0,
                        Ag[:, q * QF:(q + 1) * QF], Alu.mult, Alu.mult,
                        accum_out=m4[:, q:q + 1])
                m2a = sp.tile([128, 1], f32, name=f"m2a{g}")
                nc.vector.tensor_reduce(m2a[:], m4[:], AxX, Alu.add)
                sig = sp.tile([128, 1], f32, name=f"sig{g}")
                nc.vector.tensor_scalar(sig[:], m2a[:], 1.0 / FREE, None, Alu.mult)
                nc.scalar.activation(sig[:], sig[:], Act.Sqrt)
                t0 = sp.tile([128, 1], f32, name=f"t0{g}")
                nc.vector.tensor_tensor(t0[:], sig[:], zq_b[:], Alu.mult)

                Asmp = Ag.rearrange("p (a b) -> p a b", b=8)[:, :, 0]
                cs = sp.tile([128, 1], f32, name=f"cs{g}")
                lnr = sp.tile([128, 1], f32, name=f"lnr{g}")
                ktgt = sp.tile([128, 1], f32, name=f"ktgt{g}")
                nc.vector.tensor_scalar(ktgt[:], kk_b[:], 1.0 / 64.0, None, Alu.mult)
                rtg = sp.tile([128, 1], f32, name=f"rtg{g}")
                nc.vector.reciprocal(rtg[:], ktgt[:])
                for it in range(4):
                    nc.vector.tensor_scalar(scratch[:, :FREE // 8], Asmp, t0[:],
                                            None, Alu.is_ge, Alu.add, accum_out=cs[:])
                    nc.vector.tensor_scalar(cs[:], cs[:], 1.0, None, Alu.max)
                    nc.vector.tensor_tensor(lnr[:], cs[:], rtg[:], Alu.mult)
                    nc.vector.tensor_scalar(lnr[:], lnr[:], 0.1, 10.0, Alu.max,
                                            Alu.min)
                    nc.scalar.activation(lnr[:], lnr[:], Act.Ln)
                    nc.vector.tensor_tensor(lnr[:], lnr[:], invz2_b[:], Alu.mult)
                    nc.scalar.activation(lnr[:], lnr[:], Act.Exp)
                    nc.vector.tensor_tensor(t0[:], t0[:], lnr[:], Alu.mult)
                dma(t0_stage[g * 128:(g + 1) * 128, :], t0[:])

            # harmonize t0 across cores (ladders must be identical everywhere)
            nc.gpsimd.collective_compute(
                "AllReduce", Alu.add, replica_groups=RG,
                ins=[t0_stage[:]], outs=[t0_out[:]])

            for g in range(NG):
                Ag = A_bf[:, g * FREE:(g + 1) * FREE]
                t0 = sp.tile([128, 1], f32, name=f"t0h{g}")
                dma(t0[:], t0_out[g * 128:(g + 1) * 128, :])
                nc.vector.tensor_scalar(t0[:], t0[:], 1.0 / NCORES, None, Alu.mult)
                if DEBUG:
                    dma(dbg["dbg_t0"][g * 128:(g + 1) * 128, :], t0[:])

                tl = pw_.tile([128, NL], f32, name=f"tlad{g}")
                tl_bf = sp.tile([128, NL], bf16, name=f"tladbf{g}")
                fac = sp.tile([128, 1], f32, name=f"fac{g}")
                for j in range(NL):
                    nc.vector.tensor_scalar(fac[:], delta_b[:], float(j - NL // 2),
                                            None, Alu.mult)
                    nc.scalar.activation(fac[:], fac[:], Act.Exp)
                    nc.vector.tensor_tensor(tl[:, j:j + 1], t0[:], fac[:], Alu.mult)
                nc.vector.tensor_copy(tl_bf[:], tl[:])
                nc.vector.tensor_copy(tl[:], tl_bf[:])
                tlad_all.append(tl)
                cl = sp.tile([128, NL], f32, name=f"cl{g}")
                c4 = sp.tile([128, 4], f32, name=f"c4_{g}")
                for j in range(NL):
                    for q in range(4):
                        nc.vector.tensor_scalar(
                            scratch[:], Ag[:, q * QF:(q + 1) * QF], tl[:, j:j + 1],
                            None, Alu.is_ge, Alu.add, accum_out=c4[:, q:q + 1])
                    nc.vector.tensor_reduce(cl[:, j:j + 1], c4[:], AxX, Alu.add)
                dma(cnt_stage[g * 128:(g + 1) * 128, :], cl[:])

        nc.gpsimd.collective_compute(
            "AllReduce", Alu.add, replica_groups=RG,
            ins=[cnt_stage[:]], outs=[cnt_out[:]])

        # bracket selection (small persistent tiles)
        with tc.tile_pool(name="bracket", bufs=1) as bp:
            for g in range(NG):
                cl = bp.tile([128, NL], f32, name=f"clg{g}")
                dma(cl[:], cnt_out[g * 128:(g + 1) * 128, :])
                if DEBUG:
                    dma(dbg["dbg_cnt"][g * 128:(g + 1) * 128, 0:NL], cl[:])
                ge = bp.tile([128, NL], f32, name=f"ge{g}")
                nc.vector.tensor_scalar(ge[:], cl[:], kk_b[:], None, Alu.is_ge)
                sel = bp.tile([128, NL - 1], f32, name=f"sel{g}")
                nc.vector.tensor_scalar(sel[:], ge[:, 1:NL], -1.0, 1.0, Alu.mult,
                                        Alu.add)
                nc.vector.tensor_tensor(sel[:], sel[:], ge[:, 0:NL - 1], Alu.mult)
                t1 = bp.tile([128, 1], f32, name=f"t1_{g}")
                t2 = bp.tile([128, 1], f32, name=f"t2_{g}")
                c2 = bp.tile([128, 1], f32, name=f"c2_{g}")
                stmp = bp.tile([128, NL - 1], f32, name=f"stmp{g}")
                tl = tlad_all[g]
                nc.vector.tensor_tensor(stmp[:], sel[:], tl[:, 0:NL - 1], Alu.mult)
                nc.vector.tensor_reduce(t1[:], stmp[:], AxX, Alu.add)
                nc.vector.tensor_tensor(stmp[:], sel[:], tl[:, 1:NL], Alu.mult)
                nc.vector.tensor_reduce(t2[:], stmp[:], AxX, Alu.add)
                nc.vector.tensor_tensor(stmp[:], sel[:], cl[:, 1:NL], Alu.mult)
                nc.vector.tensor_reduce(c2[:], stmp[:], AxX, Alu.add)
                # exact fp32 count-boundary of a bf16 threshold t:
                # beta = (t + prev16(t))/2 with prev16(t) = bf16RTN(t*(1-2^-9))
                pv = bp.tile([128, 2], f32, name=f"pv{g}")
                pv_bf = bp.tile([128, 2], bf16, name=f"pvbf{g}")
                nc.vector.tensor_scalar(pv[:, 0:1], t1[:],
                                        float(1.0 - 2.0 ** -8), None, Alu.mult)
                nc.vector.tensor_scalar(pv[:, 1:2], t2[:],
                                        float(1.0 - 2.0 ** -8), None, Alu.mult)
                nc.vector.tensor_copy(pv_bf[:], pv[:])
                nc.vector.tensor_copy(pv[:], pv_bf[:])
                nc.vector.tensor_tensor(pv[:, 0:1], pv[:, 0:1], t1[:], Alu.add)
                nc.vector.tensor_tensor(pv[:, 1:2], pv[:, 1:2], t2[:], Alu.add)
                nc.vector.tensor_scalar(beta[g][0][:], pv[:, 0:1], 0.5, None,
                                        Alu.mult)
                nc.vector.tensor_scalar(beta[g][1][:], pv[:, 1:2], 0.5, None,
                                        Alu.mult)
                nc.vector.scalar_tensor_tensor(rprime[g][:], c2[:], -1.0, kk_b[:],
                                               Alu.mult, Alu.add)
                if DEBUG:
                    dma(dbg["dbg_beta"][g * 128:(g + 1) * 128, 0:1], beta[g][0][:])
                    dma(dbg["dbg_beta"][g * 128:(g + 1) * 128, 1:2], beta[g][1][:])
                    dma(dbg["dbg_beta"][g * 128:(g + 1) * 128, 2:3], c2[:])
                    dma(dbg["dbg_beta"][g * 128:(g + 1) * 128, 3:4], rprime[g][:])

        # =============== P3: band extraction (scoped pool) ===============
        with tc.tile_pool(name="p3pool", bufs=1) as xp:
            for g in range(NG):
                b1t, b2t = beta[g]
                cand = xp.tile([128, NCAND], f32, name="cand", tag="cand")
                for h in range(2):
                    A32 = xp.tile([128, HF], f32, name="A32", tag="A32")
                    Zb = xp.tile([128, HF], f32, name="Zb", tag="Zb")

                    def consume_p3(c, ps, h=h, A32=A32, g=g):
                        cc = c - h * 32
                        nc.scalar.activation(A32[:, cc * 512:(cc + 1) * 512],
                                             ps[:], Act.Abs, scale=inten[g][:])
                    flow_pass(g, consume_p3, xp, wlist=range(8 * h, 8 * h + 8))
                    nc.vector.scalar_tensor_tensor(Zb[:], A32[:], b2t[:], A32[:],
                                                   Alu.is_lt, Alu.mult)
                    ch = lambda a: a.rearrange("p (c e) -> p c e", e=64)
                    L1 = xp.tile([128, NCH], f32, name="L1", tag="L1")
                    nc.vector.tensor_reduce(L1[:], ch(Zb[:]), AxX, Alu.max)
                    L1b = L1[:].rearrange("p (c one) -> p c one", one=1).to_broadcast(
                        [128, NCH, 64])
                    nc.vector.tensor_tensor(ch(A32[:]), ch(Zb[:]), L1b, Alu.is_lt)
                    nc.vector.tensor_tensor(Zb[:], Zb[:], A32[:], Alu.mult)
                    L2 = xp.tile([128, NCH], f32, name="L2", tag="L2")
                    nc.vector.tensor_reduce(L2[:], ch(Zb[:]), AxX, Alu.max)
                    L2b = L2[:].rearrange("p (c one) -> p c one", one=1).to_broadcast(
                        [128, NCH, 64])
                    nc.vector.tensor_tensor(ch(A32[:]), ch(Zb[:]), L2b, Alu.is_lt)
                    nc.vector.tensor_tensor(Zb[:], Zb[:], A32[:], Alu.mult)
                    L3 = xp.tile([128, NCH], f32, name="L3", tag="L3")
                    nc.vector.tensor_reduce(L3[:], ch(Zb[:]), AxX, Alu.max)
                    nc.vector.scalar_tensor_tensor(L1[:], L1[:], b1t[:], L1[:],
                                                   Alu.is_ge, Alu.mult)
                    nc.vector.scalar_tensor_tensor(L2[:], L2[:], b1t[:], L2[:],
                                                   Alu.is_ge, Alu.mult)
                    nc.vector.scalar_tensor_tensor(L3[:], L3[:], b1t[:], L3[:],
                                                   Alu.is_ge, Alu.mult)
                    nc.vector.tensor_copy(cand[:, (3 * h) * NCH:(3 * h + 1) * NCH],
                                          L1[:])
                    nc.vector.tensor_copy(
                        cand[:, (3 * h + 1) * NCH:(3 * h + 2) * NCH], L2[:])
                    nc.vector.tensor_copy(
                        cand[:, (3 * h + 2) * NCH:(3 * h + 3) * NCH], L3[:])
                dma(cand_stage[g * 128:(g + 1) * 128, :], cand[:])

        nc.gpsimd.collective_compute(
            "AllGather", Alu.bypass, replica_groups=RG,
            ins=[cand_stage[:]], outs=[cand_out[:]])

        # =============== exact threshold: replicated bisection ===============
        with tc.tile_pool(name="bisect", bufs=1) as gp:
            for g in range(NG):
                G = gp.tile([128, NCORES * NCAND], f32, name="Gc", tag="Gc")
                gsc = gp.tile([128, NCORES * NCAND], f32, name="gsc", tag="gsc")
                for cidx in range(NCORES):
                    dma(G[:, cidx * NCAND:(cidx + 1) * NCAND],
                        cand_out[cidx, g * 128:(g + 1) * 128, :])
                if DEBUG and g == 0:
                    dma(dbg["dbg_cand"][0:128, :], G[:, 0:NCAND])
                lo = gp.tile([128, 1], f32, name=f"lo{g}")
                hi = gp.tile([128, 1], f32, name=f"hi{g}")
                mid = gp.tile([128, 1], f32, name=f"mid{g}")
                nmid = gp.tile([128, 1], f32, name=f"nmid{g}")
                cm = gp.tile([128, 1], f32, name=f"cm{g}")
                sl = gp.tile([128, 1], f32, name=f"sl{g}")
                nsl = gp.tile([128, 1], f32, name=f"nsl{g}")
                ta = gp.tile([128, 1], f32, name=f"ta{g}")
                tb = gp.tile([128, 1], f32, name=f"tb{g}")
                nc.vector.tensor_copy(lo[:], beta[g][0][:])
                nc.vector.tensor_copy(hi[:], beta[g][1][:])

                def upd_lohi():
                    nc.vector.tensor_scalar(sl[:], cm[:], rprime[g][:], None,
                                            Alu.is_ge)
                    nc.vector.tensor_scalar(nsl[:], sl[:], -1.0, 1.0, Alu.mult,
                                            Alu.add)
                    nc.vector.tensor_tensor(ta[:], mid[:], sl[:], Alu.mult)
                    nc.vector.tensor_tensor(tb[:], lo[:], nsl[:], Alu.mult)
                    nc.vector.tensor_tensor(lo[:], ta[:], tb[:], Alu.add)
                    nc.vector.tensor_tensor(ta[:], hi[:], sl[:], Alu.mult)
                    nc.vector.tensor_tensor(tb[:], mid[:], nsl[:], Alu.mult)
                    nc.vector.tensor_tensor(hi[:], ta[:], tb[:], Alu.add)

                for _ in range(N_BISECT):
                    nc.vector.tensor_tensor(mid[:], lo[:], hi[:], Alu.add)
                    nc.vector.tensor_scalar(mid[:], mid[:], 0.5, None, Alu.mult)
                    nc.vector.tensor_scalar(gsc[:], G[:], mid[:], None, Alu.is_ge, Alu.add,
                                            accum_out=cm[:])
                    upd_lohi()
                # cHI = count(G >= hi)
                cHI = gp.tile([128, 1], f32, name=f"cHI{g}")
                nc.vector.tensor_scalar(gsc[:], G[:], hi[:], None, Alu.is_ge, Alu.add,
                                        accum_out=cHI[:])
                # window-mask G to [lo, hi), then top-8
                nc.vector.tensor_scalar(gsc[:], G[:], lo[:], None, Alu.is_ge)
                nc.vector.scalar_tensor_tensor(G[:], G[:], hi[:], G[:], Alu.is_lt,
                                               Alu.mult)
                nc.vector.tensor_tensor(G[:], G[:], gsc[:], Alu.mult)
                W8 = gp.tile([128, 8], f32, name=f"W8{g}")
                nc.vector.max(out=W8[:], in_=G[:])
                w8s = gp.tile([128, 8], f32, name=f"w8s{g}")
                for _ in range(N_BISECT2):
                    nc.vector.tensor_tensor(mid[:], lo[:], hi[:], Alu.add)
                    nc.vector.tensor_scalar(mid[:], mid[:], 0.5, None, Alu.mult)
                    nc.vector.tensor_scalar(w8s[:], W8[:], mid[:], None, Alu.is_ge, Alu.add,
                                            accum_out=cm[:])
                    nc.vector.tensor_tensor(cm[:], cm[:], cHI[:], Alu.add)
                    upd_lohi()
                nc.vector.tensor_copy(th[g][:], lo[:])
                if DEBUG:
                    dma(dbg["dbg_th"][g * 128:(g + 1) * 128, 0:1], th[g][:])
                    dma(dbg["dbg_th"][g * 128:(g + 1) * 128, 1:2], rprime[g][:])

        if STAGE < 3:
            for g in range(NG):
                dma(out_dram[g * 128:(g + 1) * 128, :], xg[g][:])
            return nc

        # =============== P4: final masked matvec ===============
        fo_stage = pool_dram.tile([S, ISLICE], f32, name="fo_stage")
        fo_out = pool_dram.tile([NCORES, S, ISLICE], f32, name="fo_out",
                                addr_space="Shared")
        tailP = ctx.enter_context(tc.tile_pool(name="tailP", bufs=1))
        fo_full = [tailP.tile([128, D], f32, name=f"fo_full{g}") for g in range(NG)]
        with tc.tile_pool(name="p4pool", bufs=1) as fp:
            XI = []
            for g in range(NG):
                t = fp.tile([128, D], f32, name=f"XI{g}")
                nc.vector.tensor_scalar(t[:], xn[g][:], inten[g][:], None, Alu.mult)
                XI.append(t)
            for g in range(NG):
                FO = fp.tile([128, ISLICE], f32, name=f"FO{g}")

                def consume_p4(c, ps, g=g, FO=FO):
                    At = fp.tile([128, 512], f32, name="At", tag="At", bufs=3)
                    FM = fp.tile([128, 512], f32, name="FM", tag="FM", bufs=3)
                    nc.scalar.activation(At[:], ps[:], Act.Abs, scale=inten[g][:])
                    nc.vector.scalar_tensor_tensor(FM[:], At[:], th[g][:], ps[:],
                                                   Alu.is_ge, Alu.mult)
                    nc.vector.scalar_tensor_tensor(FM[:], FM[:], 1.0, XI[g][:],
                                                   Alu.mult, Alu.mult,
                                                   accum_out=FO[:, c:c + 1])
                flow_pass(g, consume_p4, fp)
                dma(fo_stage[g * 128:(g + 1) * 128, :], FO[:])

        nc.gpsimd.collective_compute(
            "AllGather", Alu.bypass, replica_groups=RG,
            ins=[fo_stage[:]], outs=[fo_out[:]])

        # =============== tail ===============
        co = [tailP.tile([128, D], f32, name=f"co{g}") for g in range(NG)]
        with tc.tile_pool(name="tail1", bufs=1) as tp:
            n2g_b = bcast_row(tp, n2_g, D, "n2g_b")
            n2b_b = bcast_row(tp, n2_b, D, "n2b_b")
            for g in range(NG):
                for cidx in range(NCORES):
                    dma(fo_full[g][:, cidx * ISLICE:(cidx + 1) * ISLICE],
                        fo_out[cidx, g * 128:(g + 1) * 128, :])
                if DEBUG:
                    dma(dbg["dbg_fo"][g * 128:(g + 1) * 128, :], fo_full[g][:])
                nc.vector.tensor_tensor(co[g][:], xg[g][:], fo_full[g][:], Alu.add)
                mean = tp.tile([128, 1], f32, name=f"mean2{g}")
                m2 = tp.tile([128, 1], f32, name=f"m2ln2{g}")
                tmp = tp.tile([128, D], f32, name=f"ln2tmp{g}", tag="tmp")
                nc.vector.tensor_reduce(mean[:], co[g][:], AxX, Alu.add)
                nc.vector.tensor_scalar(mean[:], mean[:], 1.0 / D, None, Alu.mult)
                nc.vector.tensor_scalar(tmp[:], co[g][:], mean[:], None,
                                        Alu.subtract)
                nc.vector.scalar_tensor_tensor(tmp[:], tmp[:], 1.0, tmp[:], Alu.mult,
                                               Alu.mult, accum_out=m2[:])
                nc.vector.tensor_scalar(m2[:], m2[:], 1.0 / D, 1e-5, Alu.mult,
                                        Alu.add)
                rstd = tp.tile([128, 1], f32, name=f"rstd2{g}")
                nc.scalar.activation(rstd[:], m2[:], Act.Sqrt)
                nc.vector.reciprocal(rstd[:], rstd[:])
                nc.vector.tensor_scalar(co[g][:], co[g][:], mean[:], rstd[:],
                                        Alu.subtract, Alu.mult)
                nc.vector.scalar_tensor_tensor(co[g][:], co[g][:], 1.0, n2g_b[:],
                                               Alu.mult, Alu.mult)
                nc.vector.tensor_tensor(co[g][:], co[g][:], n2b_b[:], Alu.add)

        def transposed_cols(pool, src_list, K, name):
            nk = K // 128
            tT = pool.tile([128, nk * S], f32, name=f"{name}_T")
            for g in range(NG):
                for kc in range(nk):
                    transpose_to(tT[:, kc * S + g * 128: kc * S + (g + 1) * 128],
                                 src_list[g][:, kc * 128:(kc + 1) * 128],
                                 f"{name}T{g}_{kc}")
            return lambda g, kc: tT[:, kc * S + g * 128: kc * S + (g + 1) * 128]

        def big_matmul(pool, lhsT_cols, w_dram, K, N, name, bias_dram=None,
                       const_lhsT=None, out_list=None):
            nk = K // 128
            wsb = pool.tile([128, nk * N], f32, name=f"{name}_wsb")
            for kc in range(nk):
                dma(wsb[:, kc * N:(kc + 1) * N], w_dram[kc * 128:(kc + 1) * 128, :])
            bias_b = (bcast_row(pool, bias_dram, N, f"{name}_bias")
                      if bias_dram is not None else None)
            cvec_b = None
            if const_lhsT is not None:
                cps = pool_ps.tile([1, N], f32, name="cps", tag="Tps",
                                   padded_shape=[128, 512])
                for kc in range(nk):
                    nc.tensor.matmul(cps[:1, :], const_lhsT[:, kc:kc + 1],
                                     wsb[:, kc * N:(kc + 1) * N],
                                     start=(kc == 0), stop=(kc == nk - 1))
                cvec = pool.tile([1, N], f32, name=f"{name}_cvec")
                nc.vector.tensor_copy(cvec[:], cps[:1, :])
                cvec_b = pool.tile([128, N], f32, name=f"{name}_cvecb")
                pbcast(pool, cvec_b[:], cvec[:], N, f"{name}cv")
            outs = []
            for g in range(NG):
                o = (out_list[g] if out_list is not None
                     else pool.tile([128, N], f32, name=f"{name}_o{g}"))
                for nb in range(0, N, 512):
                    nw = min(512, N - nb)
                    ps = pool_mm.tile([128, nw], f32, name="Fps", tag="Fps")
                    for kc in range(nk):
                        nc.tensor.matmul(ps[:], lhsT_cols(g, kc),
                                         wsb[:, kc * N + nb: kc * N + nb + nw],
                                         start=(kc == 0), stop=(kc == nk - 1))
                    nc.vector.tensor_copy(o[:, nb:nb + nw], ps[:])
                if bias_b is not None:
                    nc.vector.tensor_tensor(o[:], o[:], bias_b[:], Alu.add)
                if cvec_b is not None:
                    nc.vector.tensor_tensor(o[:], o[:], cvec_b[:], Alu.add)
                outs.append(o)
            return outs

        # memory-bank mean -> memvT [D,1] as 4 chunks
        with tc.tile_pool(name="tailmem", bufs=1) as mp:
            memx = mp.tile([128, 4 * D], f32, name="memx")
            for kc in range(4):
                dma(memx[:, kc * D:(kc + 1) * D],
                    memory_bank[kc * 128:(kc + 1) * 128, :])
            mem_ps = pool_ps.tile([1, D], f32, name="memps", tag="Tps",
                                  padded_shape=[128, 512])
            for kc in range(4):
                nc.tensor.matmul(mem_ps[:1, :], ones_sb[:],
                                 memx[:, kc * D:(kc + 1) * D],
                                 start=(kc == 0), stop=(kc == 3))
            memv = mp.tile([1, D], f32, name="memv")
            nc.vector.tensor_scalar(memv[:], mem_ps[:1, :], 1.0 / 512.0, None,
                                    Alu.mult)
            memvT = tailP.tile([128, 4], f32, name="memvT")
            for kc in range(4):
                transpose_to(memvT[:, kc:kc + 1], memv[:, kc * 128:(kc + 1) * 128],
                             f"memvT{kc}")

        with tc.tile_pool(name="tailA", bufs=1) as ta_:
            coT = transposed_cols(ta_, co, D, "coT")
            mh = big_matmul(ta_, coT, mem_w1, D, D, "memh", bias_dram=mem_b1,
                            const_lhsT=memvT)
            for g in range(NG):
                silu_(ta_, mh[g][:], mh[g][:], f"mh{g}")
            mhT = transposed_cols(ta_, mh, D, "mhT")
            mo = big_matmul(ta_, mhT, mem_w2, D, D, "memo", bias_dram=mem_b2)
            for g in range(NG):
                nc.vector.tensor_tensor(co[g][:], co[g][:], mo[g][:], Alu.add)

        gv = [tailP.tile([128, 4 * D], f32, name=f"gv{g}") for g in range(NG)]
        with tc.tile_pool(name="tailB", bufs=1) as tb_:
            coT2 = transposed_cols(tb_, co, D, "coT2")
            ff = big_matmul(tb_, coT2, up_w, D, 8 * D, "ff", bias_dram=up_b)
            for g in range(NG):
                silu_(tb_, gv[g][:], ff[g][:, :4 * D], f"gv{g}")
                nc.vector.tensor_tensor(gv[g][:], gv[g][:], ff[g][:, 4 * D:],
                                        Alu.mult)
        with tc.tile_pool(name="tailC", bufs=1) as tcp:
            gvT = transposed_cols(tcp, gv, 4 * D, "gvT")
            ffn = big_matmul(tcp, gvT, down_w, 4 * D, D, "ffn", bias_dram=down_b)
            for g in range(NG):
                nc.vector.tensor_tensor(ffn[g][:], ffn[g][:], co[g][:], Alu.add)
                dma(out_dram[g * 128:(g + 1) * 128, :], ffn[g][:])

    return nc


def _install_ntff_shim():
    """Reconstitute the missing antenv.axon_hooks module so
    run_bass_kernel_spmd(trace=True) can reach the axon NTFF profiler."""
    import sys
    import types

    if "antenv.axon_hooks" in sys.modules:
        return
    import antenv

    mod = types.ModuleType("antenv.axon_hooks")
    _h = [None]
    mod.set_axon_ntff_profile_hook = lambda h: _h.__setitem__(0, h)
    mod.get_axon_ntff_profile_hook = lambda: _h[0]
    sys.modules["antenv.axon_hooks"] = mod
    antenv.axon_hooks = mod
    try:
        from trn_agent_boot.trn_boot import _ntff_profile_via_ctypes

        mod.set_axon_ntff_profile_hook(
            _ntff_profile_via_ctypes("/opt/axon/libaxon_pjrt.so"))
    except Exception:
        pass


def kernel(**inputs):
    from concourse.bass_utils import run_bass_kernel_spmd
    _install_ntff_shim()

    sin, cos, qpoly = _host_constants()
    x = np.ascontiguousarray(np.asarray(inputs["x"], np.float32).reshape(S, D))
    patterns = np.ascontiguousarray(np.asarray(inputs["flow_patterns"], np.float32))

    nc = build_kernel()
    nc.finalize()

    def a(k):
        return np.ascontiguousarray(np.asarray(inputs[k], np.float32))

    def row(k):
        return np.ascontiguousarray(np.asarray(inputs[k], np.float32).reshape(1, -1))

    base = {
        "x": x,
        "sel_w1": a("sel_w1"), "sel_b1": row("sel_b1"),
        "sel_w2": a("sel_w2"), "sel_b2": row("sel_b2"),
        "win_w1": a("win_w1"), "win_b1": row("win_b1"),
        "win_w2": a("win_w2"), "win_b2": row("win_b2"),
        "int_w1": a("int_w1"), "int_b1": row("int_b1"),
        "int_w2": a("int_w2"), "int_b2": row("int_b2"),
        "mem_w1": a("mem_w1"), "mem_b1": row("mem_b1"),
        "mem_w2": a("mem_w2"), "mem_b2": row("mem_b2"),
        "memory_bank": a("memory_bank"),
        "up_w": a("up_w"), "up_b": row("up_b"),
        "down_w": a("down_w"), "down_b": row("down_b"),
        "n1_g": row("n1_g"), "n1_b": row("n1_b"),
        "n2_g": row("n2_g"), "n2_b": row("n2_b"),
        "rope_sin": sin, "rope_cos": cos,
        "qpoly": qpoly.reshape(1, 4),
    }
    in_maps = []
    for c in range(NCORES):
        m = dict(base)
        m["pat_sl"] = np.ascontiguousarray(
            patterns[:, c * ISLICE:(c + 1) * ISLICE, :].reshape(P, FREE))
        in_maps.append(m)

    trace = os.environ.get("KERNEL_TRACE", "0") == "1"
    res = run_bass_kernel_spmd(nc, in_maps, list(range(NCORES)), trace=trace)
    out0 = res.results[0]
    kernel.last_results = res.results
    kernel.last_exec_ns = getattr(res, "exec_time_ns", None)
    return out0["out"].reshape(B, S, D).astype(np.float32)


if __name__ == "__main__":
    data = np.load("/tmp/inputs.npz")
    inputs = {k: data[k] for k in data.files}
    out = kernel(**inputs)
    print("out", out.shape, float(np.abs(out).max()))



# revision 21
# speedup vs baseline: 2.4671x; 2.4671x over previous
"""Trainium2 Bass kernel for nn_EnhancedFlowLayer (topk_masking).

8 cores. Tokens on partitions (2 groups of 128); flow (i,j)-space sharded by i
across cores (64 i-rows -> 32768 elems/token/core). flow is rematerialized on
the PE per phase and never hits HBM.

Threshold strategy (replaces the exact-rank machinery of the old kernel):
 - exact per-token sigma of flow values via the pattern Gram matrix
   (tiny [16,16] AllReduce, overlapped with the preamble),
 - Gaussian quantile seed t0 = z(kk/DD) * sigma,
 - P1: one fp32r flow pass storing |F|*inten as fp16 (128KB/partition),
 - two-stage count ladder (3+3 rungs) on the fp16 data with rungs placed at
   fp16-grid midpoints, so each rung count equals the exact fp32 count at the
   midpoint; log-log interpolation to count==kk.  Two tiny AllReduces.
 - P4: fp32 flow pass, mask |F*inten| >= th on f32, masked values cast fp16,
   fp16 2x dot-accumulate against xn*inten.
One AllGather of the per-core flow_out slices, then a replicated LN2 +
memory-MLP + FFN tail (fp32r matmuls).
"""

import os
from contextlib import ExitStack

import numpy as np

B, S, D, P = 1, 256, 512, 16
MAX_SEQ = 4096
NCORES = 8
ISLICE = D // NCORES          # 64 i-rows per core
FREE = ISLICE * D             # 32768 ij elements per token per core
NG = 2                        # token groups of 128
DD = D * D
NL = 3                        # ladder rungs per stage
DLT1 = float(os.environ.get("KERNEL_DLT1", "0.01"))
DLT2 = float(os.environ.get("KERNEL_DLT2", "0.0012"))

DEBUG = os.environ.get("KERNEL_DEBUG", "0") == "1"


def _host_constants():
    pos = np.arange(S, dtype=np.float64)
    inv = 1.0 / (10000.0 ** (np.arange(0, D, 2, dtype=np.float64) / D))
    ang = pos[:, None] * inv[None, :]
    sin = np.repeat(np.sin(ang), 2, axis=-1).astype(np.float32)
    cos = np.repeat(np.cos(ang), 2, axis=-1).astype(np.float32)
    # half-normal tail quantile z(q): P(|N(0,1)| >= z) = q, cubic in ln q
    qpoly = np.array([-0.0036756, -0.06789169, -0.73664117, 0.26370117], np.float32)
    return sin, cos, qpoly


def build_kernel():
    import concourse.bass as bass
    import concourse.mybir as mybir
    from concourse import bacc, masks
    from concourse.tile import TileContext

    dt = mybir.dt
    Alu = mybir.AluOpType
    Act = mybir.ActivationFunctionType
    AxX = mybir.AxisListType.X
    f32, f16 = dt.float32, dt.float16
    f32r = dt.float32r

    nc = bacc.Bacc("TRN2", num_devices=NCORES)

    dp = nc.declare_dram_parameter
    x_in = dp("x", [S, D], f32, isOutput=False)
    pat_sl = dp("pat_sl", [P, FREE], f32, isOutput=False)
    pat_r = dp("pat_r", [P, FREE], f32r, isOutput=False)
    pat_T = dp("pat_T", [128, (FREE // 128) * P], f32, isOutput=False)
    sel_w1 = dp("sel_w1", [2 * D, 2 * P], f32, isOutput=False)
    sel_b1 = dp("sel_b1", [1, 2 * P], f32, isOutput=False)
    sel_w2 = dp("sel_w2", [2 * P, P], f32, isOutput=False)
    sel_b2 = dp("sel_b2", [1, P], f32, isOutput=False)
    win_w1 = dp("win_w1", [D, 64], f32, isOutput=False)
    win_b1 = dp("win_b1", [1, 64], f32, isOutput=False)
    win_w2 = dp("win_w2", [64, 1], f32, isOutput=False)
    win_b2 = dp("win_b2", [1, 1], f32, isOutput=False)
    int_w1 = dp("int_w1", [2 * D, 64], f32, isOutput=False)
    int_b1 = dp("int_b1", [1, 64], f32, isOutput=False)
    int_w2 = dp("int_w2", [64, 1], f32, isOutput=False)
    int_b2 = dp("int_b2", [1, 1], f32, isOutput=False)
    mem_w1 = dp("mem_w1", [2 * D, D], f32r, isOutput=False)
    mem_b1 = dp("mem_b1", [1, D], f32, isOutput=False)
    mem_w2 = dp("mem_w2", [D, D], f32r, isOutput=False)
    mem_b2 = dp("mem_b2", [1, D], f32, isOutput=False)
    memory_bank = dp("memory_bank", [512, D], f32, isOutput=False)
    up_w = dp("up_w", [D, 8 * D], f32r, isOutput=False)
    up_b = dp("up_b", [1, 8 * D], f32, isOutput=False)
    down_w = dp("down_w", [4 * D, D], f32r, isOutput=False)
    down_b = dp("down_b", [1, D], f32, isOutput=False)
    n1_g = dp("n1_g", [1, D], f32, isOutput=False)
    n1_b = dp("n1_b", [1, D], f32, isOutput=False)
    n2_g = dp("n2_g", [1, D], f32, isOutput=False)
    n2_b = dp("n2_b", [1, D], f32, isOutput=False)
    rope_sin = dp("rope_sin", [S, D], f32, isOutput=False)
    rope_cos = dp("rope_cos", [S, D], f32, isOutput=False)
    qpoly = dp("qpoly", [1, 4], f32, isOutput=False)
    out_dram = dp("out", [S, D], f32, isOutput=True)

    dbg = {}
    if DEBUG:
        for name, shape in [
            ("dbg_xn", [S, D]), ("dbg_xr", [S, D]), ("dbg_pw", [S, P]),
            ("dbg_inten", [S, 1]), ("dbg_scal", [1, 8]), ("dbg_t0", [S, 1]),
            ("dbg_cnt", [S, NL]), ("dbg_cnt2", [S, NL]), ("dbg_th", [S, 2]),
            ("dbg_fo", [S, D]), ("dbg_G", [P, P]), ("dbg_mid", [S, NL]),
        ]:
            dbg[name] = dp(name, shape, f32, isOutput=True)

    RG = [list(range(NCORES))]

    with ExitStack() as ctx:
        tc = ctx.enter_context(TileContext(nc))
        pw_ = ctx.enter_context(tc.tile_pool(name="persist", bufs=1))
        pool_mm = ctx.enter_context(tc.tile_pool(name="psumMM", bufs=6, space="PSUM"))
        pool_ps = ctx.enter_context(tc.tile_pool(name="psumT", bufs=2, space="PSUM"))
        pool_dram = ctx.enter_context(tc.tile_pool(name="dramst", bufs=1, space="DRAM"))

        def dma(dst, src):
            nc.sync.dma_start(out=dst, in_=src)

        def bcast_row(pool, src_dram_row, width, name, dtype=f32):
            t = pool.tile([128, width], dtype, name=name)
            dma(t[:], src_dram_row[:].to_broadcast([128, width]))
            return t

        identity = pw_.tile([128, 128], f32, name="identity")
        masks.make_identity(nc, identity[:])
        bc_n = [0]

        def pbcast(pool, dst_ap, src_ap, width, name):
            """broadcast [1,width] sbuf row to [128,width] via a DRAM bounce"""
            bc_n[0] += 1
            st = pool_dram.tile([1, width], f32, name=f"bc{bc_n[0]}_{name}")
            dma(st[:], src_ap)
            dma(dst_ap, st[:].to_broadcast([128, width]))

        def transpose_to(dst_ap, src_ap, name):
            p, f = src_ap.shape[0], src_ap.free_size()
            ps = pool_ps.tile([f, p], f32, name="Tps", tag="Tps",
                              padded_shape=[128, 128])
            nc.tensor.transpose(ps[:f, :p], src_ap, identity[:p, :p])
            nc.vector.tensor_copy(dst_ap, ps[:f, :p])  # rounds if dst is f32r

        def gelu_(pool, ap, name):
            e = pool.tile(list(ap.shape), f32, name=f"{name}_erf", tag="gelu_e")
            nc.scalar.activation(e[:], ap, Act.Erf, scale=float(1 / np.sqrt(2)))
            nc.vector.tensor_scalar(e[:], e[:], 1.0, 0.5, Alu.add, Alu.mult)
            nc.vector.tensor_tensor(ap, ap, e[:], Alu.mult)

        def silu_(pool, dst_ap, src_ap, name):
            sg = pool.tile(list(src_ap.shape), f32, name=f"{name}_sg", tag="silu_s")
            nc.scalar.activation(sg[:], src_ap, Act.Sigmoid)
            nc.vector.tensor_tensor(dst_ap, src_ap, sg[:], Alu.mult)

        # ---------- persistent tiles ----------
        xg = [pw_.tile([128, D], f32, name=f"xg{g}") for g in range(NG)]
        xn = [pw_.tile([128, D], f32, name=f"xn{g}") for g in range(NG)]
        pwt = [pw_.tile([P, 128], f32, name=f"pwT{g}") for g in range(NG)]
        pwt_r = [pw_.tile([P, 128], f32r, name=f"pwTr{g}") for g in range(NG)]
        pw_sb = [pw_.tile([128, P], f32, name=f"pwsb{g}") for g in range(NG)]
        inten = [pw_.tile([128, 1], f32, name=f"inten{g}") for g in range(NG)]
        kk_b = pw_.tile([128, 1], f32, name="kk_b")
        lkk_b = pw_.tile([128, 1], f32, name="lkk_b")
        zq_b = pw_.tile([128, 1], f32, name="zq_b")
        ones_sb = pw_.tile([128, 1], f32, name="ones_sb")
        nc.vector.memset(ones_sb[:], 1.0)
        t0 = [pw_.tile([128, 1], f32, name=f"t0_{g}") for g in range(NG)]
        th1 = [pw_.tile([128, 1], f32, name=f"th1_{g}") for g in range(NG)]
        th2 = [pw_.tile([128, 1], f32, name=f"th2_{g}") for g in range(NG)]
        G_sb = pw_.tile([P, P], f32, name="G_sb")

        for g in range(NG):
            dma(xg[g][:], x_in[g * 128:(g + 1) * 128, :])

        # =============== pattern Gram matrix (starts immediately;
        # AllReduce latency hides under the preamble) ===============
        G_stage = pool_dram.tile([P, P], f32, name="G_stage")
        G_out = pool_dram.tile([P, P], f32, name="G_out", addr_space="Shared")
        with tc.tile_pool(name="grampool", bufs=1) as gp0:
            G_ps = pool_ps.tile([P, P], f32, name="G_ps", tag="Tps",
                                padded_shape=[128, 128])
            NCHUNK = FREE // 128
            gTall = gp0.tile([128, NCHUNK * P], f32, name="gTall")
            dma(gTall[:], pat_T[:])
            for c in range(NCHUNK):
                nc.tensor.matmul(G_ps[:P, :P], gTall[:, c * P:(c + 1) * P],
                                 gTall[:, c * P:(c + 1) * P],
                                 start=(c == 0), stop=(c == NCHUNK - 1))
            G_loc = gp0.tile([P, P], f32, name="G_loc")
            nc.vector.tensor_copy(G_loc[:], G_ps[:P, :P])
            dma(G_stage[:], G_loc[:])
        nc.gpsimd.collective_compute(
            "AllReduce", Alu.add, replica_groups=RG,
            ins=[G_stage[:]], outs=[G_out[:]])
        dma(G_sb[:], G_out[:])
        if DEBUG:
            dma(dbg["dbg_G"][:], G_out[:])

        # =================== preamble (scoped pool) ===================
        with tc.tile_pool(name="preamble", bufs=1) as pp:
            sin_g, cos_g, xr = [], [], []
            for g in range(NG):
                t = pp.tile([128, D], f32, name=f"sin{g}")
                dma(t[:], rope_sin[g * 128:(g + 1) * 128, :])
                sin_g.append(t)
                t = pp.tile([128, D], f32, name=f"cos{g}")
                dma(t[:], rope_cos[g * 128:(g + 1) * 128, :])
                cos_g.append(t)
            n1g_b = bcast_row(pp, n1_g, D, "n1g_b")
            n1b_b = bcast_row(pp, n1_b, D, "n1b_b")

            for g in range(NG):
                mean = pp.tile([128, 1], f32, name=f"mean{g}")
                m2 = pp.tile([128, 1], f32, name=f"m2ln{g}")
                tmp = pp.tile([128, D], f32, name=f"lntmp{g}")
                nc.vector.tensor_reduce(mean[:], xg[g][:], AxX, Alu.add)
                nc.vector.tensor_scalar(mean[:], mean[:], 1.0 / D, None, Alu.mult)
                nc.vector.tensor_scalar(tmp[:], xg[g][:], mean[:], None, Alu.subtract)
                nc.vector.scalar_tensor_tensor(tmp[:], tmp[:], 1.0, tmp[:], Alu.mult,
                                               Alu.mult, accum_out=m2[:])
                nc.vector.tensor_scalar(m2[:], m2[:], 1.0 / D, 1e-5, Alu.mult, Alu.add)
                rstd = pp.tile([128, 1], f32, name=f"rstd{g}")
                nc.scalar.activation(rstd[:], m2[:], Act.Sqrt)
                nc.vector.reciprocal(rstd[:], rstd[:])
                nc.vector.tensor_scalar(xn[g][:], xg[g][:], mean[:], rstd[:],
                                        Alu.subtract, Alu.mult)
                nc.vector.scalar_tensor_tensor(xn[g][:], xn[g][:], 1.0, n1g_b[:],
                                               Alu.mult, Alu.mult)
                nc.vector.tensor_tensor(xn[g][:], xn[g][:], n1b_b[:], Alu.add)
                t_xr = pp.tile([128, D], f32, name=f"xr{g}")
                rot = pp.tile([128, D], f32, name=f"rot{g}")
                ev = lambda a: a.rearrange("p (a two) -> p a two", two=2)[:, :, 0]
                od = lambda a: a.rearrange("p (a two) -> p a two", two=2)[:, :, 1]
                nc.vector.tensor_scalar(ev(rot[:]), od(xn[g][:]), -1.0, None, Alu.mult)
                nc.vector.tensor_copy(od(rot[:]), ev(xn[g][:]))
                nc.vector.tensor_tensor(rot[:], rot[:], sin_g[g][:], Alu.mult)
                nc.vector.scalar_tensor_tensor(t_xr[:], xn[g][:], 1.0, cos_g[g][:],
                                               Alu.mult, Alu.mult)
                nc.vector.tensor_tensor(t_xr[:], t_xr[:], rot[:], Alu.add)
                xr.append(t_xr)

            # ctx = mean over tokens
            ctx_ps = pool_ps.tile([1, D], f32, name="ctx_ps", tag="Tps",
                                  padded_shape=[128, 512])
            for g in range(NG):
                nc.tensor.matmul(ctx_ps[:1, :], ones_sb[:], xr[g][:],
                                 start=(g == 0), stop=(g == NG - 1))
            ctx_row = pp.tile([1, D], f32, name="ctx_row")
            nc.vector.tensor_scalar(ctx_row[:], ctx_ps[:1, :], 1.0 / S, None, Alu.mult)

            xrT = pp.tile([128, 4 * S], f32, name="xrT")
            for g in range(NG):
                for kc in range(4):
                    transpose_to(xrT[:, kc * S + g * 128: kc * S + (g + 1) * 128],
                                 xr[g][:, kc * 128:(kc + 1) * 128], f"xrT{g}{kc}")
            ctxT = pp.tile([128, 4], f32, name="ctxT")
            for kc in range(4):
                transpose_to(ctxT[:, kc:kc + 1], ctx_row[:, kc * 128:(kc + 1) * 128],
                             f"ctxT{kc}")

            def mlp_head(w1, b1, w2, b2, h1_dim, h2_dim, name):
                w1a = pp.tile([128, 4 * h1_dim], f32, name=f"{name}_w1a")
                w1b = pp.tile([128, 4 * h1_dim], f32, name=f"{name}_w1b")
                for kc in range(4):
                    dma(w1a[:, kc * h1_dim:(kc + 1) * h1_dim],
                        w1[kc * 128:(kc + 1) * 128, :])
                    dma(w1b[:, kc * h1_dim:(kc + 1) * h1_dim],
                        w1[D + kc * 128: D + (kc + 1) * 128, :])
                b1_b = bcast_row(pp, b1, h1_dim, f"{name}_b1b")
                w2_sb = pp.tile([h1_dim, h2_dim], f32, name=f"{name}_w2sb")
                dma(w2_sb[:], w2[:])
                b2_b = bcast_row(pp, b2, h2_dim, f"{name}_b2b")
                v1_ps = pool_ps.tile([1, h1_dim], f32, name="v1ps", tag="Tps",
                                     padded_shape=[128, 128])
                for kc in range(4):
                    nc.tensor.matmul(v1_ps[:1, :], ctxT[:, kc:kc + 1],
                                     w1b[:, kc * h1_dim:(kc + 1) * h1_dim],
                                     start=(kc == 0), stop=(kc == 3))
                v1 = pp.tile([1, h1_dim], f32, name=f"{name}_v1")
                nc.vector.tensor_copy(v1[:], v1_ps[:1, :])
                v1_b = pp.tile([128, h1_dim], f32, name=f"{name}_v1b")
                pbcast(pp, v1_b[:], v1[:], h1_dim, f"{name}v1")
                outs = []
                for g in range(NG):
                    h1_ps = pool_ps.tile([128, h1_dim], f32, name="h1ps", tag="Tps",
                                         padded_shape=[128, 128])
                    for kc in range(4):
                        nc.tensor.matmul(
                            h1_ps[:], xrT[:, kc * S + g * 128: kc * S + (g + 1) * 128],
                            w1a[:, kc * h1_dim:(kc + 1) * h1_dim],
                            start=(kc == 0), stop=(kc == 3))
                    h1 = pp.tile([128, h1_dim], f32, name=f"{name}_h1_{g}")
                    nc.vector.tensor_tensor(h1[:], h1_ps[:], v1_b[:], Alu.add)
                    nc.vector.tensor_tensor(h1[:], h1[:], b1_b[:], Alu.add)
                    gelu_(pp, h1[:], f"{name}g{g}")
                    h1T = pp.tile([h1_dim, 128], f32, name=f"{name}_h1T_{g}")
                    transpose_to(h1T[:], h1[:], f"{name}h1T{g}")
                    h2_ps = pool_ps.tile([128, h2_dim], f32, name="h2ps", tag="Tps",
                                         padded_shape=[128, 128])
                    nc.tensor.matmul(h2_ps[:], h1T[:], w2_sb[:], start=True, stop=True)
                    h2 = pp.tile([128, h2_dim], f32, name=f"{name}_h2_{g}")
                    nc.vector.tensor_tensor(h2[:], h2_ps[:], b2_b[:], Alu.add)
                    outs.append(h2)
                return outs

            sel_h2 = mlp_head(sel_w1, sel_b1, sel_w2, sel_b2, 2 * P, P, "sel")
            int_h2 = mlp_head(int_w1, int_b1, int_w2, int_b2, 64, 1, "intm")

            for g in range(NG):
                mx = pp.tile([128, 1], f32, name=f"selmx{g}")
                nc.vector.tensor_reduce(mx[:], sel_h2[g][:], AxX, Alu.max)
                nc.vector.tensor_scalar(sel_h2[g][:], sel_h2[g][:], mx[:], None,
                                        Alu.subtract)
                nc.scalar.activation(sel_h2[g][:], sel_h2[g][:], Act.Exp)
                sm = pp.tile([128, 1], f32, name=f"selsm{g}")
                nc.vector.tensor_reduce(sm[:], sel_h2[g][:], AxX, Alu.add)
                rs = pp.tile([128, 1], f32, name=f"selrs{g}")
                nc.vector.reciprocal(rs[:], sm[:])
                nc.vector.tensor_scalar(pw_sb[g][:], sel_h2[g][:], rs[:], None,
                                        Alu.mult)
                nc.scalar.activation(inten[g][:], int_h2[g][:], Act.Sigmoid)
                transpose_to(pwt[g][:], pw_sb[g][:], f"pwT{g}")
                nc.vector.tensor_copy(pwt_r[g][:], pwt[g][:])
                if DEBUG:
                    dma(dbg["dbg_pw"][g * 128:(g + 1) * 128, :], pw_sb[g][:])

            # window scalar -> kk, z
            winw1_sb = pp.tile([128, 4 * 64], f32, name="winw1_sb")
            for kc in range(4):
                dma(winw1_sb[:, kc * 64:(kc + 1) * 64],
                    win_w1[kc * 128:(kc + 1) * 128, :])
            wh1_ps = pool_ps.tile([1, 64], f32, name="wh1ps", tag="Tps",
                                  padded_shape=[128, 128])
            for kc in range(4):
                nc.tensor.matmul(wh1_ps[:1, :], ctxT[:, kc:kc + 1],
                                 winw1_sb[:, kc * 64:(kc + 1) * 64],
                                 start=(kc == 0), stop=(kc == 3))
            wh1 = pp.tile([1, 64], f32, name="wh1")
            wb1_sb = pp.tile([1, 64], f32, name="wb1_sb")
            dma(wb1_sb[:], win_b1[:])
            nc.vector.tensor_tensor(wh1[:], wh1_ps[:1, :], wb1_sb[:], Alu.add)
            gelu_(pp, wh1[:], "wh1g")
            wh1T = pp.tile([64, 1], f32, name="wh1T")
            transpose_to(wh1T[:], wh1[:], "wh1T")
            winw2_sb = pp.tile([64, 1], f32, name="winw2_sb")
            dma(winw2_sb[:], win_w2[:])
            win_ps = pool_ps.tile([1, 1], f32, name="winps", tag="Tps",
                                  padded_shape=[128, 128])
            nc.tensor.matmul(win_ps[:1, :1], wh1T[:], winw2_sb[:], start=True,
                             stop=True)
            winv = pp.tile([1, 1], f32, name="winv")
            wb2_sb = pp.tile([1, 1], f32, name="wb2_sb")
            dma(wb2_sb[:], win_b2[:])
            nc.vector.tensor_tensor(winv[:], win_ps[:1, :1], wb2_sb[:], Alu.add)
            nc.scalar.activation(winv[:], winv[:], Act.Sigmoid)
            nc.vector.tensor_scalar(winv[:], winv[:], float(MAX_SEQ - 256), 256.0,
                                    Alu.mult, Alu.add)
            kkf = pp.tile([1, 1], f32, name="kkf")
            nc.vector.tensor_scalar(kkf[:], winv[:], 0.1 / MAX_SEQ * DD, None,
                                    Alu.mult)
            # floor() robust to the f32->i32 convert rounding mode
            ki = pp.tile([1, 1], dt.int32, name="ki")
            nc.vector.tensor_copy(ki[:], kkf[:])
            kf2 = pp.tile([1, 1], f32, name="kf2")
            nc.vector.tensor_copy(kf2[:], ki[:])
            kgt = pp.tile([1, 1], f32, name="kgt")
            nc.vector.tensor_tensor(kgt[:], kf2[:], kkf[:], Alu.is_gt)
            nc.vector.tensor_tensor(kkf[:], kf2[:], kgt[:], Alu.subtract)
            nc.vector.tensor_scalar(kkf[:], kkf[:], 1.0, None, Alu.max)

            qp = pp.tile([1, 4], f32, name="qp")
            dma(qp[:], qpoly[:])
            u = pp.tile([1, 1], f32, name="qu")
            nc.vector.tensor_scalar(u[:], kkf[:], 1.0 / DD, None, Alu.mult)
            nc.scalar.activation(u[:], u[:], Act.Ln)
            zq = pp.tile([1, 1], f32, name="zq")
            nc.vector.tensor_scalar(zq[:], qp[:, 0:1], u[:], qp[:, 1:2], Alu.mult,
                                    Alu.add)
            nc.vector.tensor_scalar(zq[:], zq[:], u[:], qp[:, 2:3], Alu.mult, Alu.add)
            nc.vector.tensor_scalar(zq[:], zq[:], u[:], qp[:, 3:4], Alu.mult, Alu.add)
            pbcast(pp, kk_b[:], kkf[:], 1, "kk")
            pbcast(pp, zq_b[:], zq[:], 1, "zq")
            nc.scalar.activation(lkk_b[:], kk_b[:], Act.Ln)

            # sigma per token via Gram: q2 = pw^T G pw ; t0 = z*sqrt(q2/DD)*inten
            for g in range(NG):
                sig_ps = pool_ps.tile([128, P], f32, name="sigps", tag="Tps",
                                      padded_shape=[128, 128])
                nc.tensor.matmul(sig_ps[:], pwt[g][:], G_sb[:], start=True, stop=True)
                q2 = pp.tile([128, 1], f32, name=f"q2_{g}")
                scr = pp.tile([128, P], f32, name=f"q2scr{g}", tag="q2scr")
                nc.vector.scalar_tensor_tensor(scr[:], sig_ps[:], 1.0, pw_sb[g][:],
                                               Alu.mult, Alu.mult, accum_out=q2[:])
                sig = pp.tile([128, 1], f32, name=f"sig{g}")
                nc.scalar.activation(sig[:], q2[:], Act.Sqrt, scale=float(1.0 / DD))
                nc.vector.tensor_tensor(sig[:], sig[:], zq_b[:], Alu.mult)
                nc.vector.tensor_tensor(t0[g][:], sig[:], inten[g][:], Alu.mult)
                if DEBUG:
                    dma(dbg["dbg_t0"][g * 128:(g + 1) * 128, :], t0[g][:])

            if DEBUG:
                for g in range(NG):
                    dma(dbg["dbg_xn"][g * 128:(g + 1) * 128, :], xn[g][:])
                    dma(dbg["dbg_xr"][g * 128:(g + 1) * 128, :], xr[g][:])
                    dma(dbg["dbg_inten"][g * 128:(g + 1) * 128, :], inten[g][:])
                dma(dbg["dbg_scal"][:, 0:1], kkf[:])
                dma(dbg["dbg_scal"][:, 1:2], winv[:])
                dma(dbg["dbg_scal"][:, 2:3], zq[:])

        # =========== helper: stream patterns & rematerialize F ===========
        def flow_pass(g, consume, pat_pool, pat_dram, pwt_t, pdt):
            """consume(c, psum_ap) for each 512-chunk c (i_loc = c) of group g."""
            for w in range(16):
                patw = pat_pool.tile([P, 2048], pdt, name="patw", tag="patw", bufs=3)
                dma(patw[:], pat_dram[:, w * 2048:(w + 1) * 2048])
                for m in range(4):
                    c = w * 4 + m
                    ps = pool_mm.tile([128, 512], f32, name="Fps", tag="Fps")
                    nc.tensor.matmul(ps[:], pwt_t[g][:],
                                     patw[:, m * 512:(m + 1) * 512],
                                     start=True, stop=True)
                    consume(c, ps)

        # =============== ladder helpers ===============
        # g*(1-1.25*2^-11) lies 0.625..1.25 fp16-ULP below grid point g for any
        # mantissa, so RTN-to-fp16 lands exactly on the previous grid point.
        PREV16 = float(1.0 - 1.25 * 2.0 ** -11)

        def build_rungs(pool, center, scale_consts, g, name):
            """rungs at fp16-grid midpoints around center; returns (mids, lmids)
            mids: [128, NL] f32 thresholds; lmids: [128, NL] ln(mid)"""
            mids = pool.tile([128, NL], f32, name=f"{name}_mid{g}")
            lmids = pool.tile([128, NL], f32, name=f"{name}_lmid{g}")
            graw = pool.tile([128, NL], f32, name=f"{name}_graw{g}")
            gf = pool.tile([128, NL], f32, name=f"{name}_gf{g}")
            g16 = pool.tile([128, NL], f16, name=f"{name}_g16{g}")
            gdec = pool.tile([128, NL], f16, name=f"{name}_gdec{g}")
            for j in range(NL):
                nc.vector.tensor_scalar(graw[:, j:j + 1], center[:],
                                        float(scale_consts[j]), None, Alu.mult)
            nc.vector.tensor_copy(g16[:], graw[:])              # rtn to fp16 grid
            nc.vector.tensor_copy(gf[:], g16[:])                # grid point, f32
            nc.vector.tensor_scalar(graw[:], gf[:], PREV16, None, Alu.mult)
            nc.vector.tensor_copy(gdec[:], graw[:])             # prev grid point
            nc.vector.tensor_copy(mids[:], gdec[:])
            nc.vector.tensor_tensor(mids[:], mids[:], gf[:], Alu.add)
            nc.vector.tensor_scalar(mids[:], mids[:], 0.5, None, Alu.mult)
            nc.scalar.activation(lmids[:], mids[:], Act.Ln)
            return mids, lmids

        def count_rungs(pool, Ag, mids, cl, scratch, g, name):
            """cl[:, j] = # (Ag >= mids[:, j]) for each rung (two halves)."""
            HW_ = FREE // 2
            ch = pool.tile([128, 2], f32, name=f"{name}_ch{g}", tag="cnt_ch")
            for j in range(NL):
                for h in range(2):
                    nc.vector.tensor_scalar(scratch[:], Ag[:, h * HW_:(h + 1) * HW_],
                                            mids[:, j:j + 1], None, Alu.is_ge,
                                            Alu.add, accum_out=ch[:, h:h + 1])
                nc.vector.tensor_reduce(cl[:, j:j + 1], ch[:], AxX, Alu.add)

        def interp_th(pool, cl, lmids, th_out, g, name):
            """log-log piecewise-linear interp of count->kk over NL=3 rungs."""
            lc = pool.tile([128, NL], f32, name=f"{name}_lc{g}")
            nc.vector.tensor_scalar(lc[:], cl[:], 1.0, None, Alu.max)
            nc.scalar.activation(lc[:], lc[:], Act.Ln)
            shi = pool.tile([128, 1], f32, name=f"{name}_shi{g}")
            nc.vector.tensor_scalar(shi[:], cl[:, 1:2], kk_b[:], None, Alu.is_ge)
            slo = pool.tile([128, 1], f32, name=f"{name}_slo{g}")
            nc.vector.tensor_scalar(slo[:], shi[:], -1.0, 1.0, Alu.mult, Alu.add)

            def blend(dst, a_hi, a_lo, tmp):
                nc.vector.tensor_tensor(dst, a_hi, shi[:], Alu.mult)
                nc.vector.tensor_tensor(tmp, a_lo, slo[:], Alu.mult)
                nc.vector.tensor_tensor(dst, dst, tmp, Alu.add)

            tmp = pool.tile([128, 1], f32, name=f"{name}_tmp{g}")
            num = pool.tile([128, 1], f32, name=f"{name}_num{g}")
            den = pool.tile([128, 1], f32, name=f"{name}_den{g}")
            base = pool.tile([128, 1], f32, name=f"{name}_base{g}")
            dl = pool.tile([128, 1], f32, name=f"{name}_dl{g}")
            d01 = pool.tile([128, 1], f32, name=f"{name}_d01{g}")
            d12 = pool.tile([128, 1], f32, name=f"{name}_d12{g}")
            # num = (lc[seg_lo_idx] - lkk)
            nc.vector.tensor_scalar(d01[:], lc[:, 1:2], lkk_b[:], None, Alu.subtract)
            nc.vector.tensor_scalar(d12[:], lc[:, 0:1], lkk_b[:], None, Alu.subtract)
            blend(num[:], d01[:], d12[:], tmp[:])
            # den = (lc[lo] - lc[hi])
            nc.vector.tensor_scalar(d01[:], lc[:, 1:2], lc[:, 2:3], None, Alu.subtract)
            nc.vector.tensor_scalar(d12[:], lc[:, 0:1], lc[:, 1:2], None, Alu.subtract)
            blend(den[:], d01[:], d12[:], tmp[:])
            nc.vector.tensor_scalar(den[:], den[:], 1e-5, None, Alu.max)
            # base / dl
            blend(base[:], lmids[:, 1:2], lmids[:, 0:1], tmp[:])
            nc.vector.tensor_scalar(d01[:], lmids[:, 2:3], lmids[:, 1:2], None,
                                    Alu.subtract)
            nc.vector.tensor_scalar(d12[:], lmids[:, 1:2], lmids[:, 0:1], None,
                                    Alu.subtract)
            blend(dl[:], d01[:], d12[:], tmp[:])
            nc.vector.reciprocal(den[:], den[:])
            nc.vector.tensor_tensor(num[:], num[:], den[:], Alu.mult)
            nc.vector.tensor_tensor(num[:], num[:], dl[:], Alu.mult)
            nc.vector.tensor_tensor(base[:], base[:], num[:], Alu.add)
            nc.scalar.activation(th_out[:], base[:], Act.Exp)

        # =============== P1: |F| -> fp16 + two-stage ladder ===============
        t_stage = pool_dram.tile([S, NL], f32, name="t_stage")
        t_out = pool_dram.tile([S, NL], f32, name="t_out", addr_space="Shared")
        t2_stage = pool_dram.tile([S, NL], f32, name="t2_stage")
        t2_out = pool_dram.tile([S, NL], f32, name="t2_out", addr_space="Shared")

        e1 = [float(np.exp(DLT1 * (j - 1))) for j in range(NL)]
        e2 = [float(np.exp(DLT2 * (j - 1))) for j in range(NL)]

        with tc.tile_pool(name="selpool", bufs=1) as sp:
            A16 = sp.tile([128, NG * FREE], f16, name="A16")
            scratch = sp.tile([128, FREE // 2], f16, name="scratch")

            for g in range(NG):
                def consume_p1(c, ps, g=g):
                    nc.scalar.activation(
                        A16[:, g * FREE + c * 512: g * FREE + (c + 1) * 512],
                        ps[:], Act.Abs, scale=inten[g][:])
                flow_pass(g, consume_p1, sp, pat_r, pwt_r, f32r)

            # stage 1
            lm1 = []
            for g in range(NG):
                mids, lmids = build_rungs(sp, t0[g], e1, g, "s1")
                lm1.append(lmids)
                cl = sp.tile([128, NL], f32, name=f"cl1_{g}")
                count_rungs(sp, A16[:, g * FREE:(g + 1) * FREE], mids, cl,
                            scratch, g, "s1")
                dma(t_stage[g * 128:(g + 1) * 128, :], cl[:])
                if DEBUG:
                    dma(dbg["dbg_mid"][g * 128:(g + 1) * 128, :], mids[:])
            nc.gpsimd.collective_compute(
                "AllReduce", Alu.add, replica_groups=RG,
                ins=[t_stage[:]], outs=[t_out[:]])
            for g in range(NG):
                cl = sp.tile([128, NL], f32, name=f"cl1g_{g}")
                dma(cl[:], t_out[g * 128:(g + 1) * 128, :])
                if DEBUG:
                    dma(dbg["dbg_cnt"][g * 128:(g + 1) * 128, :], cl[:])
                interp_th(sp, cl, lm1[g], th1[g][:], g, "i1")

            # stage 2
            lm2 = []
            for g in range(NG):
                mids, lmids = build_rungs(sp, th1[g], e2, g, "s2")
                lm2.append(lmids)
                cl = sp.tile([128, NL], f32, name=f"cl2_{g}")
                count_rungs(sp, A16[:, g * FREE:(g + 1) * FREE], mids, cl,
                            scratch, g, "s2")
                dma(t2_stage[g * 128:(g + 1) * 128, :], cl[:])
            nc.gpsimd.collective_compute(
                "AllReduce", Alu.add, replica_groups=RG,
                ins=[t2_stage[:]], outs=[t2_out[:]])
            for g in range(NG):
                cl = sp.tile([128, NL], f32, name=f"cl2g_{g}")
                dma(cl[:], t2_out[g * 128:(g + 1) * 128, :])
                if DEBUG:
                    dma(dbg["dbg_cnt2"][g * 128:(g + 1) * 128, :], cl[:])
                interp_th(sp, cl, lm2[g], th2[g][:], g, "i2")
                if DEBUG:
                    dma(dbg["dbg_th"][g * 128:(g + 1) * 128, 0:1], th1[g][:])
                    dma(dbg["dbg_th"][g * 128:(g + 1) * 128, 1:2], th2[g][:])

        # =============== P4: final masked matvec (fp32 pass) ===============
        fo_stage = pool_dram.tile([S, ISLICE], f32, name="fo_stage")
        fo_out = pool_dram.tile([NCORES, S, ISLICE], f32, name="fo_out",
                                addr_space="Shared")
        tailP = ctx.enter_context(tc.tile_pool(name="tailP", bufs=1))
        fo_full = [tailP.tile([128, D], f32, name=f"fo_full{g}") for g in range(NG)]
        with tc.tile_pool(name="p4pool", bufs=1) as fp:
            XI16 = []
            for g in range(NG):
                t = fp.tile([128, D], f16, name=f"XI16_{g}")
                nc.vector.tensor_scalar(t[:], xn[g][:], inten[g][:], None, Alu.mult)
                XI16.append(t)
            for g in range(NG):
                FO = fp.tile([128, ISLICE], f32, name=f"FO{g}")

                def consume_p4(c, ps, g=g, FO=FO):
                    At = fp.tile([128, 512], f32, name="At", tag="At", bufs=3)
                    FM = fp.tile([128, 512], f16, name="FM", tag="FM", bufs=3)
                    sc16 = fp.tile([128, 512], f16, name="sc16", tag="sc16", bufs=3)
                    nc.scalar.activation(At[:], ps[:], Act.Abs, scale=inten[g][:])
                    nc.vector.scalar_tensor_tensor(FM[:], At[:], th2[g][:], ps[:],
                                                   Alu.is_ge, Alu.mult)
                    nc.vector.scalar_tensor_tensor(sc16[:], FM[:], 1.0, XI16[g][:],
                                                   Alu.mult, Alu.mult,
                                                   accum_out=FO[:, c:c + 1])
                flow_pass(g, consume_p4, fp, pat_sl, pwt, f32)
                dma(fo_stage[g * 128:(g + 1) * 128, :], FO[:])

        nc.gpsimd.collective_compute(
            "AllGather", Alu.bypass, replica_groups=RG,
            ins=[fo_stage[:]], outs=[fo_out[:]])

        # =============== tail ===============
        co = [tailP.tile([128, D], f32, name=f"co{g}") for g in range(NG)]
        with tc.tile_pool(name="tail1", bufs=1) as tp:
            n2g_b = bcast_row(tp, n2_g, D, "n2g_b")
            n2b_b = bcast_row(tp, n2_b, D, "n2b_b")
            for g in range(NG):
                for cidx in range(NCORES):
                    dma(fo_full[g][:, cidx * ISLICE:(cidx + 1) * ISLICE],
                        fo_out[cidx, g * 128:(g + 1) * 128, :])
                if DEBUG:
                    dma(dbg["dbg_fo"][g * 128:(g + 1) * 128, :], fo_full[g][:])
                nc.vector.tensor_tensor(co[g][:], xg[g][:], fo_full[g][:], Alu.add)
                mean = tp.tile([128, 1], f32, name=f"mean2{g}")
                m2 = tp.tile([128, 1], f32, name=f"m2ln2{g}")
                tmp = tp.tile([128, D], f32, name=f"ln2tmp{g}", tag="tmp")
                nc.vector.tensor_reduce(mean[:], co[g][:], AxX, Alu.add)
                nc.vector.tensor_scalar(mean[:], mean[:], 1.0 / D, None, Alu.mult)
                nc.vector.tensor_scalar(tmp[:], co[g][:], mean[:], None,
                                        Alu.subtract)
                nc.vector.scalar_tensor_tensor(tmp[:], tmp[:], 1.0, tmp[:], Alu.mult,
                                               Alu.mult, accum_out=m2[:])
                nc.vector.tensor_scalar(m2[:], m2[:], 1.0 / D, 1e-5, Alu.mult,
                                        Alu.add)
                rstd = tp.tile([128, 1], f32, name=f"rstd2{g}")
                nc.scalar.activation(rstd[:], m2[:], Act.Sqrt)
                nc.vector.reciprocal(rstd[:], rstd[:])
                nc.vector.tensor_scalar(co[g][:], co[g][:], mean[:], rstd[:],
                                        Alu.subtract, Alu.mult)
                nc.vector.scalar_tensor_tensor(co[g][:], co[g][:], 1.0, n2g_b[:],
                                               Alu.mult, Alu.mult)
                nc.vector.tensor_tensor(co[g][:], co[g][:], n2b_b[:], Alu.add)

        def transposed_cols(pool, src_list, K, name):
            nk = K // 128
            tT = pool.tile([128, nk * S], f32r, name=f"{name}_T")
            for g in range(NG):
                for kc in range(nk):
                    transpose_to(tT[:, kc * S + g * 128: kc * S + (g + 1) * 128],
                                 src_list[g][:, kc * 128:(kc + 1) * 128],
                                 f"{name}T{g}_{kc}")
            return lambda g, kc: tT[:, kc * S + g * 128: kc * S + (g + 1) * 128]

        def big_matmul(pool, lhsT_cols, w_dram, K, N, name, bias_dram=None,
                       const_lhsT=None, out_list=None):
            nk = K // 128
            wsb = pool.tile([128, nk * N], f32r, name=f"{name}_wsb")
            for kc in range(nk):
                dma(wsb[:, kc * N:(kc + 1) * N], w_dram[kc * 128:(kc + 1) * 128, :])
            bias_b = (bcast_row(pool, bias_dram, N, f"{name}_bias")
                      if bias_dram is not None else None)
            cvec_b = None
            if const_lhsT is not None:
                cps = pool_ps.tile([1, N], f32, name="cps", tag="Tps",
                                   padded_shape=[128, 512])
                for kc in range(nk):
                    nc.tensor.matmul(cps[:1, :], const_lhsT[:, kc:kc + 1],
                                     wsb[:, kc * N:(kc + 1) * N],
                                     start=(kc == 0), stop=(kc == nk - 1))
                cvec = pool.tile([1, N], f32, name=f"{name}_cvec")
                nc.vector.tensor_copy(cvec[:], cps[:1, :])
                cvec_b = pool.tile([128, N], f32, name=f"{name}_cvecb")
                pbcast(pool, cvec_b[:], cvec[:], N, f"{name}cv")
            outs = []
            for g in range(NG):
                o = (out_list[g] if out_list is not None
                     else pool.tile([128, N], f32, name=f"{name}_o{g}"))
                for nb in range(0, N, 512):
                    nw = min(512, N - nb)
                    ps = pool_mm.tile([128, nw], f32, name="Fps", tag="Fps")
                    for kc in range(nk):
                        nc.tensor.matmul(ps[:], lhsT_cols(g, kc),
                                         wsb[:, kc * N + nb: kc * N + nb + nw],
                                         start=(kc == 0), stop=(kc == nk - 1))
                    nc.vector.tensor_copy(o[:, nb:nb + nw], ps[:])
                if bias_b is not None:
                    nc.vector.tensor_tensor(o[:], o[:], bias_b[:], Alu.add)
                if cvec_b is not None:
                    nc.vector.tensor_tensor(o[:], o[:], cvec_b[:], Alu.add)
                outs.append(o)
            return outs

        # memory-bank mean -> memvT [D,1] as 4 chunks
        with tc.tile_pool(name="tailmem", bufs=1) as mp:
            memx = mp.tile([128, 4 * D], f32, name="memx")
            for kc in range(4):
                dma(memx[:, kc * D:(kc + 1) * D],
                    memory_bank[kc * 128:(kc + 1) * 128, :])
            mem_ps = pool_ps.tile([1, D], f32, name="memps", tag="Tps",
                                  padded_shape=[128, 512])
            for kc in range(4):
                nc.tensor.matmul(mem_ps[:1, :], ones_sb[:],
                                 memx[:, kc * D:(kc + 1) * D],
                                 start=(kc == 0), stop=(kc == 3))
            memv = mp.tile([1, D], f32, name="memv")
            nc.vector.tensor_scalar(memv[:], mem_ps[:1, :], 1.0 / 512.0, None,
                                    Alu.mult)
            memvT = tailP.tile([128, 4], f32r, name="memvT")
            for kc in range(4):
                transpose_to(memvT[:, kc:kc + 1], memv[:, kc * 128:(kc + 1) * 128],
                             f"memvT{kc}")

        with tc.tile_pool(name="tailA", bufs=1) as ta_:
            coT = transposed_cols(ta_, co, D, "coT")
            mh = big_matmul(ta_, coT, mem_w1, D, D, "memh", bias_dram=mem_b1,
                            const_lhsT=memvT)
            for g in range(NG):
                silu_(ta_, mh[g][:], mh[g][:], f"mh{g}")
            mhT = transposed_cols(ta_, mh, D, "mhT")
            mo = big_matmul(ta_, mhT, mem_w2, D, D, "memo", bias_dram=mem_b2)
            for g in range(NG):
                nc.vector.tensor_tensor(co[g][:], co[g][:], mo[g][:], Alu.add)

        gv = [tailP.tile([128, 4 * D], f32, name=f"gv{g}") for g in range(NG)]
        with tc.tile_pool(name="tailB", bufs=1) as tb_:
            coT2 = transposed_cols(tb_, co, D, "coT2")
            ff = big_matmul(tb_, coT2, up_w, D, 8 * D, "ff", bias_dram=up_b)
            for g in range(NG):
                silu_(tb_, gv[g][:], ff[g][:, :4 * D], f"gv{g}")
                nc.vector.tensor_tensor(gv[g][:], gv[g][:], ff[g][:, 4 * D:],
                                        Alu.mult)
        with tc.tile_pool(name="tailC", bufs=1) as tcp:
            gvT = transposed_cols(tcp, gv, 4 * D, "gvT")
            ffn = big_matmul(tcp, gvT, down_w, 4 * D, D, "ffn", bias_dram=down_b)
            for g in range(NG):
                nc.vector.tensor_tensor(ffn[g][:], ffn[g][:], co[g][:], Alu.add)
                dma(out_dram[g * 128:(g + 1) * 128, :], ffn[g][:])

    return nc


def _install_ntff_shim():
    """Reconstitute the missing antenv.axon_hooks module so
    run_bass_kernel_spmd(trace=True) can reach the axon NTFF profiler."""
    import sys
    import types

    if "antenv.axon_hooks" in sys.modules:
        return
    import antenv

    mod = types.ModuleType("antenv.axon_hooks")
    _h = [None]
    mod.set_axon_ntff_profile_hook = lambda h: _h.__setitem__(0, h)
    mod.get_axon_ntff_profile_hook = lambda: _h[0]
    sys.modules["antenv.axon_hooks"] = mod
    antenv.axon_hooks = mod
    try:
        from trn_agent_boot.trn_boot import _ntff_profile_via_ctypes

        mod.set_axon_ntff_profile_hook(
            _ntff_profile_via_ctypes("/opt/axon/libaxon_pjrt.so"))
    except Exception:
        pass


def kernel(**inputs):
    from concourse.bass_utils import run_bass_kernel_spmd
    _install_ntff_shim()

    sin, cos, qpoly = _host_constants()
    x = np.ascontiguousarray(np.asarray(inputs["x"], np.float32).reshape(S, D))
    patterns = np.ascontiguousarray(np.asarray(inputs["flow_patterns"], np.float32))

    nc = build_kernel()
    nc.finalize()

    def a(k):
        return np.ascontiguousarray(np.asarray(inputs[k], np.float32))

    def row(k):
        return np.ascontiguousarray(np.asarray(inputs[k], np.float32).reshape(1, -1))

    base = {
        "x": x,
        "sel_w1": a("sel_w1"), "sel_b1": row("sel_b1"),
        "sel_w2": a("sel_w2"), "sel_b2": row("sel_b2"),
        "win_w1": a("win_w1"), "win_b1": row("win_b1"),
        "win_w2": a("win_w2"), "win_b2": row("win_b2"),
        "int_w1": a("int_w1"), "int_b1": row("int_b1"),
        "int_w2": a("int_w2"), "int_b2": row("int_b2"),
        "mem_w1": a("mem_w1"), "mem_b1": row("mem_b1"),
        "mem_w2": a("mem_w2"), "mem_b2": row("mem_b2"),
        "memory_bank": a("memory_bank"),
        "up_w": a("up_w"), "up_b": row("up_b"),
        "down_w": a("down_w"), "down_b": row("down_b"),
        "n1_g": row("n1_g"), "n1_b": row("n1_b"),
        "n2_g": row("n2_g"), "n2_b": row("n2_b"),
        "rope_sin": sin, "rope_cos": cos,
        "qpoly": qpoly.reshape(1, 4),
    }
    in_maps = []
    for c in range(NCORES):
        m = dict(base)
        psl = np.ascontiguousarray(
            patterns[:, c * ISLICE:(c + 1) * ISLICE, :].reshape(P, FREE))
        m["pat_sl"] = psl
        m["pat_r"] = psl
        # [FREE, P] -> [128, (FREE/128)*P]: partition p holds rows p, p+128, ...
        m["pat_T"] = np.ascontiguousarray(
            psl.T.reshape(FREE // 128, 128, P).transpose(1, 0, 2).reshape(
                128, (FREE // 128) * P))
        in_maps.append(m)

    trace = os.environ.get("KERNEL_TRACE", "0") == "1"
    res = run_bass_kernel_spmd(nc, in_maps, list(range(NCORES)), trace=trace)
    out0 = res.results[0]
    kernel.last_results = res.results
    kernel.last_exec_ns = getattr(res, "exec_time_ns", None)
    return out0["out"].reshape(B, S, D).astype(np.float32)


if __name__ == "__main__":
    data = np.load("/tmp/inputs.npz")
    inputs = {k: data[k] for k in data.files}
    out = kernel(**inputs)
    print("out", out.shape, float(np.abs(out).max()))


# revision 33
# speedup vs baseline: 3.2246x; 1.3071x over previous
"""Trainium2 Bass kernel for nn_EnhancedFlowLayer (topk_masking).

8 cores. Tokens on partitions (2 groups of 128); flow (i,j)-space sharded by i
across cores (64 i-rows -> 32768 elems/token/core). flow is rematerialized on
the PE per phase and never hits HBM.

Threshold strategy (replaces the exact-rank machinery of the old kernel):
 - exact per-token sigma of flow values via the pattern Gram matrix
   (tiny [16,16] AllReduce, overlapped with the preamble),
 - Gaussian quantile seed t0 = z(kk/DD) * sigma,
 - P1: one fp32r flow pass storing |F|*inten as fp16 (128KB/partition),
 - two-stage count ladder (3+3 rungs) on the fp16 data with rungs placed at
   fp16-grid midpoints, so each rung count equals the exact fp32 count at the
   midpoint; log-log interpolation to count==kk.  Two tiny AllReduces.
 - P4: fp32 flow pass, mask |F*inten| >= th on f32, masked values cast fp16,
   fp16 2x dot-accumulate against xn*inten.
One AllGather of the per-core flow_out slices, then a replicated LN2 +
memory-MLP + FFN tail (fp32r matmuls).
"""

import os
from contextlib import ExitStack

import numpy as np

B, S, D, P = 1, 256, 512, 16
MAX_SEQ = 4096
NCORES = 8
ISLICE = D // NCORES          # 64 i-rows per core
FREE = ISLICE * D             # 32768 ij elements per token per core
NG = 2                        # token groups of 128
DD = D * D
NL1 = 2                       # stage-1 ladder rungs
NL2 = 3                       # stage-2 ladder rungs
DLT1 = float(os.environ.get("KERNEL_DLT1", "0.01"))
DLT2 = float(os.environ.get("KERNEL_DLT2", "0.0015"))
QW = FREE // 4                # ladder count quarter width (8192)

DEBUG = os.environ.get("KERNEL_DEBUG", "0") == "1"


def _host_constants():
    pos = np.arange(S, dtype=np.float64)
    inv = 1.0 / (10000.0 ** (np.arange(0, D, 2, dtype=np.float64) / D))
    ang = pos[:, None] * inv[None, :]
    sin = np.repeat(np.sin(ang), 2, axis=-1).astype(np.float32)
    cos = np.repeat(np.cos(ang), 2, axis=-1).astype(np.float32)
    # half-normal tail quantile z(q): P(|N(0,1)| >= z) = q, cubic in ln q
    qpoly = np.array([-0.0036756, -0.06789169, -0.73664117, 0.26370117], np.float32)
    return sin, cos, qpoly


def build_kernel():
    import concourse.bass as bass
    import concourse.mybir as mybir
    from concourse import bacc, masks
    from concourse.tile import TileContext

    dt = mybir.dt
    Alu = mybir.AluOpType
    Act = mybir.ActivationFunctionType
    AxX = mybir.AxisListType.X
    f32, f16 = dt.float32, dt.float16
    f32r = dt.float32r

    nc = bacc.Bacc("TRN2", num_devices=NCORES)

    bf16 = dt.bfloat16
    dp = nc.declare_dram_parameter
    x_in = dp("x", [S, D], f32, isOutput=False)
    pat_r = dp("pat_r", [P, FREE], f32r, isOutput=False)
    pat_hi = dp("pat_hi", [P, FREE], bf16, isOutput=False)
    pat_lo = dp("pat_lo", [P, FREE], bf16, isOutput=False)
    pat_T = dp("pat_T", [128, (FREE // 128) * P], f32, isOutput=False)
    sel_w1 = dp("sel_w1", [2 * D, 2 * P], f32, isOutput=False)
    sel_b1 = dp("sel_b1", [1, 2 * P], f32, isOutput=False)
    sel_w2 = dp("sel_w2", [2 * P, P], f32, isOutput=False)
    sel_b2 = dp("sel_b2", [1, P], f32, isOutput=False)
    win_w1 = dp("win_w1", [D, 64], f32, isOutput=False)
    win_b1 = dp("win_b1", [1, 64], f32, isOutput=False)
    win_w2 = dp("win_w2", [64, 1], f32, isOutput=False)
    win_b2 = dp("win_b2", [1, 1], f32, isOutput=False)
    int_w1 = dp("int_w1", [2 * D, 64], f32, isOutput=False)
    int_b1 = dp("int_b1", [1, 64], f32, isOutput=False)
    int_w2 = dp("int_w2", [64, 1], f32, isOutput=False)
    int_b2 = dp("int_b2", [1, 1], f32, isOutput=False)
    mem_w1 = dp("mem_w1", [2 * D, D], f32r, isOutput=False)
    mem_b1 = dp("mem_b1", [1, D], f32, isOutput=False)
    mem_w2 = dp("mem_w2", [D, D], f32r, isOutput=False)
    mem_b2 = dp("mem_b2", [1, D], f32, isOutput=False)
    memory_bank = dp("memory_bank", [512, D], f32, isOutput=False)
    up_w = dp("up_w", [D, 8 * D], f32r, isOutput=False)
    up_b = dp("up_b", [1, 8 * D], f32, isOutput=False)
    down_w = dp("down_w", [4 * D, D], f32r, isOutput=False)
    down_b = dp("down_b", [1, D], f32, isOutput=False)
    n1_g = dp("n1_g", [1, D], f32, isOutput=False)
    n1_b = dp("n1_b", [1, D], f32, isOutput=False)
    n2_g = dp("n2_g", [1, D], f32, isOutput=False)
    n2_b = dp("n2_b", [1, D], f32, isOutput=False)
    rope_sin = dp("rope_sin", [S, D], f32, isOutput=False)
    rope_cos = dp("rope_cos", [S, D], f32, isOutput=False)
    qpoly = dp("qpoly", [1, 4], f32, isOutput=False)
    out_dram = dp("out", [S, D], f32, isOutput=True)

    dbg = {}
    if DEBUG:
        for name, shape in [
            ("dbg_xn", [S, D]), ("dbg_xr", [S, D]), ("dbg_pw", [S, P]),
            ("dbg_inten", [S, 1]), ("dbg_scal", [1, 8]), ("dbg_t0", [S, 1]),
            ("dbg_cnt", [S, 3]), ("dbg_cnt2", [S, 3]), ("dbg_th", [S, 2]),
            ("dbg_fo", [S, D]), ("dbg_G", [P, P]), ("dbg_mid", [S, 3]),
        ]:
            dbg[name] = dp(name, shape, f32, isOutput=True)

    RG = [list(range(NCORES))]

    with ExitStack() as ctx:
        tc = ctx.enter_context(TileContext(nc))
        pw_ = ctx.enter_context(tc.tile_pool(name="persist", bufs=1))
        pool_mm = ctx.enter_context(tc.tile_pool(name="psumMM", bufs=6, space="PSUM"))
        pool_ps = ctx.enter_context(tc.tile_pool(name="psumT", bufs=2, space="PSUM"))
        pool_dram = ctx.enter_context(tc.tile_pool(name="dramst", bufs=1, space="DRAM"))

        def dma(dst, src):
            nc.sync.dma_start(out=dst, in_=src)

        def bcast_row(pool, src_dram_row, width, name, dtype=f32):
            t = pool.tile([128, width], dtype, name=name)
            dma(t[:], src_dram_row[:].to_broadcast([128, width]))
            return t

        identity = pw_.tile([128, 128], f32, name="identity")
        masks.make_identity(nc, identity[:])
        bc_n = [0]

        def pbcast(pool, dst_ap, src_ap, width, name):
            """broadcast [1,width] sbuf row to [128,width] via a DRAM bounce"""
            bc_n[0] += 1
            st = pool_dram.tile([1, width], f32, name=f"bc{bc_n[0]}_{name}")
            dma(st[:], src_ap)
            dma(dst_ap, st[:].to_broadcast([128, width]))

        def transpose_to(dst_ap, src_ap, name):
            p, f = src_ap.shape[0], src_ap.free_size()
            ps = pool_ps.tile([f, p], f32, name="Tps", tag="Tps",
                              padded_shape=[128, 128])
            nc.tensor.transpose(ps[:f, :p], src_ap, identity[:p, :p])
            nc.vector.tensor_copy(dst_ap, ps[:f, :p])  # rounds if dst is f32r

        def gelu_(pool, ap, name):
            e = pool.tile(list(ap.shape), f32, name=f"{name}_erf", tag="gelu_e")
            nc.scalar.activation(e[:], ap, Act.Erf, scale=float(1 / np.sqrt(2)))
            nc.vector.tensor_scalar(e[:], e[:], 1.0, 0.5, Alu.add, Alu.mult)
            nc.vector.tensor_tensor(ap, ap, e[:], Alu.mult)

        def silu_(pool, dst_ap, src_ap, name):
            sg = pool.tile(list(src_ap.shape), f32, name=f"{name}_sg", tag="silu_s")
            nc.scalar.activation(sg[:], src_ap, Act.Sigmoid)
            nc.vector.tensor_tensor(dst_ap, src_ap, sg[:], Alu.mult)

        # ---------- persistent tiles ----------
        xg = [pw_.tile([128, D], f32, name=f"xg{g}") for g in range(NG)]
        xn = [pw_.tile([128, D], f32, name=f"xn{g}") for g in range(NG)]
        pwt = [pw_.tile([P, 128], f32, name=f"pwT{g}") for g in range(NG)]
        pwt_r = [pw_.tile([P, 128], f32r, name=f"pwTr{g}") for g in range(NG)]
        pwt_hi = [pw_.tile([P, 128], bf16, name=f"pwTh{g}") for g in range(NG)]
        pwt_lo = [pw_.tile([P, 128], bf16, name=f"pwTl{g}") for g in range(NG)]
        pw_sb = [pw_.tile([128, P], f32, name=f"pwsb{g}") for g in range(NG)]
        inten = [pw_.tile([128, 1], f32, name=f"inten{g}") for g in range(NG)]
        kk_b = pw_.tile([128, 1], f32, name="kk_b")
        lkk_b = pw_.tile([128, 1], f32, name="lkk_b")
        zq_b = pw_.tile([128, 1], f32, name="zq_b")
        ones_sb = pw_.tile([128, 1], f32, name="ones_sb")
        nc.vector.memset(ones_sb[:], 1.0)
        t0 = [pw_.tile([128, 1], f32, name=f"t0_{g}") for g in range(NG)]
        th1 = [pw_.tile([128, 1], f32, name=f"th1_{g}") for g in range(NG)]
        th2 = [pw_.tile([128, 1], f32, name=f"th2_{g}") for g in range(NG)]
        G_sb = pw_.tile([P, P], f32, name="G_sb")

        for g in range(NG):
            dma(xg[g][:], x_in[g * 128:(g + 1) * 128, :])

        # =============== pattern Gram matrix (starts immediately;
        # AllReduce latency hides under the preamble) ===============
        G_stage = pool_dram.tile([P, P], f32, name="G_stage")
        G_out = pool_dram.tile([P, P], f32, name="G_out", addr_space="Shared")
        with tc.tile_pool(name="grampool", bufs=1) as gp0:
            G_ps = pool_ps.tile([P, P], f32, name="G_ps", tag="Tps",
                                padded_shape=[128, 128])
            NCHUNK = FREE // 128
            gTall = gp0.tile([128, NCHUNK * P], f32, name="gTall")
            dma(gTall[:], pat_T[:])
            for c in range(NCHUNK):
                nc.tensor.matmul(G_ps[:P, :P], gTall[:, c * P:(c + 1) * P],
                                 gTall[:, c * P:(c + 1) * P],
                                 start=(c == 0), stop=(c == NCHUNK - 1))
            G_loc = gp0.tile([P, P], f32, name="G_loc")
            nc.vector.tensor_copy(G_loc[:], G_ps[:P, :P])
            dma(G_stage[:], G_loc[:])
        nc.gpsimd.collective_compute(
            "AllReduce", Alu.add, replica_groups=RG,
            ins=[G_stage[:]], outs=[G_out[:]])
        dma(G_sb[:], G_out[:])
        if DEBUG:
            dma(dbg["dbg_G"][:], G_out[:])

        # =================== preamble (scoped pool) ===================
        with tc.tile_pool(name="preamble", bufs=1) as pp:
            sin_g, cos_g, xr = [], [], []
            for g in range(NG):
                t = pp.tile([128, D], f32, name=f"sin{g}")
                dma(t[:], rope_sin[g * 128:(g + 1) * 128, :])
                sin_g.append(t)
                t = pp.tile([128, D], f32, name=f"cos{g}")
                dma(t[:], rope_cos[g * 128:(g + 1) * 128, :])
                cos_g.append(t)
            n1g_b = bcast_row(pp, n1_g, D, "n1g_b")
            n1b_b = bcast_row(pp, n1_b, D, "n1b_b")

            for g in range(NG):
                mean = pp.tile([128, 1], f32, name=f"mean{g}")
                m2 = pp.tile([128, 1], f32, name=f"m2ln{g}")
                tmp = pp.tile([128, D], f32, name=f"lntmp{g}")
                nc.vector.tensor_reduce(mean[:], xg[g][:], AxX, Alu.add)
                nc.vector.tensor_scalar(mean[:], mean[:], 1.0 / D, None, Alu.mult)
                nc.vector.tensor_scalar(tmp[:], xg[g][:], mean[:], None, Alu.subtract)
                nc.vector.scalar_tensor_tensor(tmp[:], tmp[:], 1.0, tmp[:], Alu.mult,
                                               Alu.mult, accum_out=m2[:])
                nc.vector.tensor_scalar(m2[:], m2[:], 1.0 / D, 1e-5, Alu.mult, Alu.add)
                rstd = pp.tile([128, 1], f32, name=f"rstd{g}")
                nc.scalar.activation(rstd[:], m2[:], Act.Sqrt)
                nc.vector.reciprocal(rstd[:], rstd[:])
                nc.vector.tensor_scalar(xn[g][:], xg[g][:], mean[:], rstd[:],
                                        Alu.subtract, Alu.mult)
                nc.vector.scalar_tensor_tensor(xn[g][:], xn[g][:], 1.0, n1g_b[:],
                                               Alu.mult, Alu.mult)
                nc.vector.tensor_tensor(xn[g][:], xn[g][:], n1b_b[:], Alu.add)
                t_xr = pp.tile([128, D], f32, name=f"xr{g}")
                rot = pp.tile([128, D], f32, name=f"rot{g}")
                ev = lambda a: a.rearrange("p (a two) -> p a two", two=2)[:, :, 0]
                od = lambda a: a.rearrange("p (a two) -> p a two", two=2)[:, :, 1]
                nc.vector.tensor_scalar(ev(rot[:]), od(xn[g][:]), -1.0, None, Alu.mult)
                nc.vector.tensor_copy(od(rot[:]), ev(xn[g][:]))
                nc.vector.tensor_tensor(rot[:], rot[:], sin_g[g][:], Alu.mult)
                nc.vector.scalar_tensor_tensor(t_xr[:], xn[g][:], 1.0, cos_g[g][:],
                                               Alu.mult, Alu.mult)
                nc.vector.tensor_tensor(t_xr[:], t_xr[:], rot[:], Alu.add)
                xr.append(t_xr)

            # ctx = mean over tokens
            ctx_ps = pool_ps.tile([1, D], f32, name="ctx_ps", tag="Tps",
                                  padded_shape=[128, 512])
            for g in range(NG):
                nc.tensor.matmul(ctx_ps[:1, :], ones_sb[:], xr[g][:],
                                 start=(g == 0), stop=(g == NG - 1))
            ctx_row = pp.tile([1, D], f32, name="ctx_row")
            nc.vector.tensor_scalar(ctx_row[:], ctx_ps[:1, :], 1.0 / S, None, Alu.mult)

            xrT = pp.tile([128, 4 * S], f32, name="xrT")
            for g in range(NG):
                for kc in range(4):
                    transpose_to(xrT[:, kc * S + g * 128: kc * S + (g + 1) * 128],
                                 xr[g][:, kc * 128:(kc + 1) * 128], f"xrT{g}{kc}")
            ctxT = pp.tile([128, 4], f32, name="ctxT")
            for kc in range(4):
                transpose_to(ctxT[:, kc:kc + 1], ctx_row[:, kc * 128:(kc + 1) * 128],
                             f"ctxT{kc}")

            def mlp_head(w1, b1, w2, b2, h1_dim, h2_dim, name):
                w1a = pp.tile([128, 4 * h1_dim], f32, name=f"{name}_w1a")
                w1b = pp.tile([128, 4 * h1_dim], f32, name=f"{name}_w1b")
                for kc in range(4):
                    dma(w1a[:, kc * h1_dim:(kc + 1) * h1_dim],
                        w1[kc * 128:(kc + 1) * 128, :])
                    dma(w1b[:, kc * h1_dim:(kc + 1) * h1_dim],
                        w1[D + kc * 128: D + (kc + 1) * 128, :])
                b1_b = bcast_row(pp, b1, h1_dim, f"{name}_b1b")
                w2_sb = pp.tile([h1_dim, h2_dim], f32, name=f"{name}_w2sb")
                dma(w2_sb[:], w2[:])
                b2_b = bcast_row(pp, b2, h2_dim, f"{name}_b2b")
                v1_ps = pool_ps.tile([1, h1_dim], f32, name="v1ps", tag="Tps",
                                     padded_shape=[128, 128])
                for kc in range(4):
                    nc.tensor.matmul(v1_ps[:1, :], ctxT[:, kc:kc + 1],
                                     w1b[:, kc * h1_dim:(kc + 1) * h1_dim],
                                     start=(kc == 0), stop=(kc == 3))
                v1 = pp.tile([1, h1_dim], f32, name=f"{name}_v1")
                nc.vector.tensor_copy(v1[:], v1_ps[:1, :])
                v1_b = pp.tile([128, h1_dim], f32, name=f"{name}_v1b")
                pbcast(pp, v1_b[:], v1[:], h1_dim, f"{name}v1")
                outs = []
                for g in range(NG):
                    h1_ps = pool_ps.tile([128, h1_dim], f32, name="h1ps", tag="Tps",
                                         padded_shape=[128, 128])
                    for kc in range(4):
                        nc.tensor.matmul(
                            h1_ps[:], xrT[:, kc * S + g * 128: kc * S + (g + 1) * 128],
                            w1a[:, kc * h1_dim:(kc + 1) * h1_dim],
                            start=(kc == 0), stop=(kc == 3))
                    h1 = pp.tile([128, h1_dim], f32, name=f"{name}_h1_{g}")
                    nc.vector.tensor_tensor(h1[:], h1_ps[:], v1_b[:], Alu.add)
                    nc.vector.tensor_tensor(h1[:], h1[:], b1_b[:], Alu.add)
                    gelu_(pp, h1[:], f"{name}g{g}")
                    h1T = pp.tile([h1_dim, 128], f32, name=f"{name}_h1T_{g}")
                    transpose_to(h1T[:], h1[:], f"{name}h1T{g}")
                    h2_ps = pool_ps.tile([128, h2_dim], f32, name="h2ps", tag="Tps",
                                         padded_shape=[128, 128])
                    nc.tensor.matmul(h2_ps[:], h1T[:], w2_sb[:], start=True, stop=True)
                    h2 = pp.tile([128, h2_dim], f32, name=f"{name}_h2_{g}")
                    nc.vector.tensor_tensor(h2[:], h2_ps[:], b2_b[:], Alu.add)
                    outs.append(h2)
                return outs

            sel_h2 = mlp_head(sel_w1, sel_b1, sel_w2, sel_b2, 2 * P, P, "sel")
            int_h2 = mlp_head(int_w1, int_b1, int_w2, int_b2, 64, 1, "intm")

            for g in range(NG):
                mx = pp.tile([128, 1], f32, name=f"selmx{g}")
                nc.vector.tensor_reduce(mx[:], sel_h2[g][:], AxX, Alu.max)
                nc.vector.tensor_scalar(sel_h2[g][:], sel_h2[g][:], mx[:], None,
                                        Alu.subtract)
                nc.scalar.activation(sel_h2[g][:], sel_h2[g][:], Act.Exp)
                sm = pp.tile([128, 1], f32, name=f"selsm{g}")
                nc.vector.tensor_reduce(sm[:], sel_h2[g][:], AxX, Alu.add)
                rs = pp.tile([128, 1], f32, name=f"selrs{g}")
                nc.vector.reciprocal(rs[:], sm[:])
                nc.vector.tensor_scalar(pw_sb[g][:], sel_h2[g][:], rs[:], None,
                                        Alu.mult)
                nc.scalar.activation(inten[g][:], int_h2[g][:], Act.Sigmoid)
                transpose_to(pwt[g][:], pw_sb[g][:], f"pwT{g}")
                nc.vector.tensor_copy(pwt_r[g][:], pwt[g][:])
                # bf16 hi/lo split of pw for the precise P4 matmul
                nc.vector.tensor_copy(pwt_hi[g][:], pwt[g][:])
                hi32 = pp.tile([P, 128], f32, name=f"hi32_{g}")
                nc.vector.tensor_copy(hi32[:], pwt_hi[g][:])
                nc.vector.tensor_tensor(hi32[:], pwt[g][:], hi32[:], Alu.subtract)
                nc.vector.tensor_copy(pwt_lo[g][:], hi32[:])
                if DEBUG:
                    dma(dbg["dbg_pw"][g * 128:(g + 1) * 128, :], pw_sb[g][:])

            # window scalar -> kk, z
            winw1_sb = pp.tile([128, 4 * 64], f32, name="winw1_sb")
            for kc in range(4):
                dma(winw1_sb[:, kc * 64:(kc + 1) * 64],
                    win_w1[kc * 128:(kc + 1) * 128, :])
            wh1_ps = pool_ps.tile([1, 64], f32, name="wh1ps", tag="Tps",
                                  padded_shape=[128, 128])
            for kc in range(4):
                nc.tensor.matmul(wh1_ps[:1, :], ctxT[:, kc:kc + 1],
                                 winw1_sb[:, kc * 64:(kc + 1) * 64],
                                 start=(kc == 0), stop=(kc == 3))
            wh1 = pp.tile([1, 64], f32, name="wh1")
            wb1_sb = pp.tile([1, 64], f32, name="wb1_sb")
            dma(wb1_sb[:], win_b1[:])
            nc.vector.tensor_tensor(wh1[:], wh1_ps[:1, :], wb1_sb[:], Alu.add)
            gelu_(pp, wh1[:], "wh1g")
            wh1T = pp.tile([64, 1], f32, name="wh1T")
            transpose_to(wh1T[:], wh1[:], "wh1T")
            winw2_sb = pp.tile([64, 1], f32, name="winw2_sb")
            dma(winw2_sb[:], win_w2[:])
            win_ps = pool_ps.tile([1, 1], f32, name="winps", tag="Tps",
                                  padded_shape=[128, 128])
            nc.tensor.matmul(win_ps[:1, :1], wh1T[:], winw2_sb[:], start=True,
                             stop=True)
            winv = pp.tile([1, 1], f32, name="winv")
            wb2_sb = pp.tile([1, 1], f32, name="wb2_sb")
            dma(wb2_sb[:], win_b2[:])
            nc.vector.tensor_tensor(winv[:], win_ps[:1, :1], wb2_sb[:], Alu.add)
            nc.scalar.activation(winv[:], winv[:], Act.Sigmoid)
            nc.vector.tensor_scalar(winv[:], winv[:], float(MAX_SEQ - 256), 256.0,
                                    Alu.mult, Alu.add)
            kkf = pp.tile([1, 1], f32, name="kkf")
            nc.vector.tensor_scalar(kkf[:], winv[:], 0.1 / MAX_SEQ * DD, None,
                                    Alu.mult)
            # floor() robust to the f32->i32 convert rounding mode
            ki = pp.tile([1, 1], dt.int32, name="ki")
            nc.vector.tensor_copy(ki[:], kkf[:])
            kf2 = pp.tile([1, 1], f32, name="kf2")
            nc.vector.tensor_copy(kf2[:], ki[:])
            kgt = pp.tile([1, 1], f32, name="kgt")
            nc.vector.tensor_tensor(kgt[:], kf2[:], kkf[:], Alu.is_gt)
            nc.vector.tensor_tensor(kkf[:], kf2[:], kgt[:], Alu.subtract)
            nc.vector.tensor_scalar(kkf[:], kkf[:], 1.0, None, Alu.max)

            qp = pp.tile([1, 4], f32, name="qp")
            dma(qp[:], qpoly[:])
            u = pp.tile([1, 1], f32, name="qu")
            nc.vector.tensor_scalar(u[:], kkf[:], 1.0 / DD, None, Alu.mult)
            nc.scalar.activation(u[:], u[:], Act.Ln)
            zq = pp.tile([1, 1], f32, name="zq")
            nc.vector.tensor_scalar(zq[:], qp[:, 0:1], u[:], qp[:, 1:2], Alu.mult,
                                    Alu.add)
            nc.vector.tensor_scalar(zq[:], zq[:], u[:], qp[:, 2:3], Alu.mult, Alu.add)
            nc.vector.tensor_scalar(zq[:], zq[:], u[:], qp[:, 3:4], Alu.mult, Alu.add)
            pbcast(pp, kk_b[:], kkf[:], 1, "kk")
            pbcast(pp, zq_b[:], zq[:], 1, "zq")
            nc.scalar.activation(lkk_b[:], kk_b[:], Act.Ln)

            # sigma per token via Gram: q2 = pw^T G pw ; t0 = z*sqrt(q2/DD)*inten
            for g in range(NG):
                sig_ps = pool_ps.tile([128, P], f32, name="sigps", tag="Tps",
                                      padded_shape=[128, 128])
                nc.tensor.matmul(sig_ps[:], pwt[g][:], G_sb[:], start=True, stop=True)
                q2 = pp.tile([128, 1], f32, name=f"q2_{g}")
                scr = pp.tile([128, P], f32, name=f"q2scr{g}", tag="q2scr")
                nc.vector.scalar_tensor_tensor(scr[:], sig_ps[:], 1.0, pw_sb[g][:],
                                               Alu.mult, Alu.mult, accum_out=q2[:])
                sig = pp.tile([128, 1], f32, name=f"sig{g}")
                nc.scalar.activation(sig[:], q2[:], Act.Sqrt, scale=float(1.0 / DD))
                nc.vector.tensor_tensor(sig[:], sig[:], zq_b[:], Alu.mult)
                nc.vector.tensor_tensor(t0[g][:], sig[:], inten[g][:], Alu.mult)
                if DEBUG:
                    dma(dbg["dbg_t0"][g * 128:(g + 1) * 128, :], t0[g][:])

            if DEBUG:
                for g in range(NG):
                    dma(dbg["dbg_xn"][g * 128:(g + 1) * 128, :], xn[g][:])
                    dma(dbg["dbg_xr"][g * 128:(g + 1) * 128, :], xr[g][:])
                    dma(dbg["dbg_inten"][g * 128:(g + 1) * 128, :], inten[g][:])
                dma(dbg["dbg_scal"][:, 0:1], kkf[:])
                dma(dbg["dbg_scal"][:, 1:2], winv[:])
                dma(dbg["dbg_scal"][:, 2:3], zq[:])

        # =========== helpers: stream patterns & rematerialize F ===========
        def flow_pass_r(g, consume, pat_pool):
            """fp32r pass (counting-grade precision)."""
            for w in range(16):
                patw = pat_pool.tile([P, 2048], f32r, name="patw", tag="patw",
                                     bufs=3)
                dma(patw[:], pat_r[:, w * 2048:(w + 1) * 2048])
                for m in range(4):
                    c = w * 4 + m
                    ps = pool_mm.tile([128, 512], f32, name="Fps", tag="Fps")
                    nc.tensor.matmul(ps[:], pwt_r[g][:],
                                     patw[:, m * 512:(m + 1) * 512],
                                     start=True, stop=True)
                    consume(c, ps)

        def flow_pass_hl(g, consume, pat_pool):
            """3-term bf16 split pass: hi*hi + hi*lo + lo*hi (~2^-16 precision,
            runs at full bf16 PE rate unlike fp32's half-rate 2-slice form)."""
            for w in range(16):
                pwh = pat_pool.tile([P, 2048], bf16, name="pwh", tag="pwh", bufs=3)
                pwl = pat_pool.tile([P, 2048], bf16, name="pwl", tag="pwl", bufs=3)
                dma(pwh[:], pat_hi[:, w * 2048:(w + 1) * 2048])
                dma(pwl[:], pat_lo[:, w * 2048:(w + 1) * 2048])
                for m in range(4):
                    c = w * 4 + m
                    ps = pool_mm.tile([128, 512], f32, name="Fps", tag="Fps")
                    nc.tensor.matmul(ps[:], pwt_hi[g][:],
                                     pwh[:, m * 512:(m + 1) * 512],
                                     start=True, stop=False)
                    nc.tensor.matmul(ps[:], pwt_hi[g][:],
                                     pwl[:, m * 512:(m + 1) * 512],
                                     start=False, stop=False)
                    nc.tensor.matmul(ps[:], pwt_lo[g][:],
                                     pwh[:, m * 512:(m + 1) * 512],
                                     start=False, stop=True)
                    consume(c, ps)

        # =============== ladder helpers ===============
        # g*(1-1.25*2^-11) lies 0.625..1.25 fp16-ULP below grid point g for any
        # mantissa, so RTN-to-fp16 lands exactly on the previous grid point.
        PREV16 = float(1.0 - 1.25 * 2.0 ** -11)

        def build_rungs(pool, center, scale_consts, g, name):
            """rungs at fp16-grid midpoints around center; returns (mids, lmids)"""
            nl = len(scale_consts)
            mids = pool.tile([128, nl], f32, name=f"{name}_mid{g}")
            lmids = pool.tile([128, nl], f32, name=f"{name}_lmid{g}")
            nmids = pool.tile([128, nl], f32, name=f"{name}_nmid{g}")
            graw = pool.tile([128, nl], f32, name=f"{name}_graw{g}")
            gf = pool.tile([128, nl], f32, name=f"{name}_gf{g}")
            g16 = pool.tile([128, nl], f16, name=f"{name}_g16{g}")
            gdec = pool.tile([128, nl], f16, name=f"{name}_gdec{g}")
            for j in range(nl):
                nc.vector.tensor_scalar(graw[:, j:j + 1], center[:],
                                        float(scale_consts[j]), None, Alu.mult)
            nc.vector.tensor_copy(g16[:], graw[:])              # rtn to fp16 grid
            nc.vector.tensor_copy(gf[:], g16[:])                # grid point, f32
            nc.vector.tensor_scalar(graw[:], gf[:], PREV16, None, Alu.mult)
            nc.vector.tensor_copy(gdec[:], graw[:])             # prev grid point
            nc.vector.tensor_copy(mids[:], gdec[:])
            nc.vector.tensor_tensor(mids[:], mids[:], gf[:], Alu.add)
            nc.vector.tensor_scalar(mids[:], mids[:], 0.5, None, Alu.mult)
            nc.vector.tensor_scalar(nmids[:], mids[:], -1.0, None, Alu.mult)
            nc.scalar.activation(lmids[:], mids[:], Act.Ln)
            return mids, lmids, nmids

        def count_rungs(pool, Ag, mids, nmids, nl, cl, scr_v, scr_s, g, name):
            """cl[:, j] = # (Ag >= mids[:, j]); quarters split scalar/vector.

            Scalar quarters use Sign(A - mid) accumulated: S = #ge - #lt, so
            #ge = 0.5*S + QW/2 per quarter (mids sit strictly between fp16
            grid points, so A - mid never equals 0)."""
            ch = pool.tile([128, 5], f32, name=f"{name}_ch{g}", tag="cnt_ch")
            for j in range(nl):
                for q in range(4):
                    Aq = Ag[:, q * QW:(q + 1) * QW]
                    if q < 2:
                        nc.scalar.activation(scr_s[:], Aq, Act.Sign,
                                             bias=nmids[:, j:j + 1],
                                             accum_out=ch[:, q:q + 1])
                    else:
                        nc.vector.tensor_scalar(scr_v[:], Aq, mids[:, j:j + 1],
                                                None, Alu.is_ge, Alu.add,
                                                accum_out=ch[:, q:q + 1])
                nc.vector.tensor_reduce(cl[:, j:j + 1], ch[:, 0:2], AxX, Alu.add)
                nc.vector.tensor_scalar(cl[:, j:j + 1], cl[:, j:j + 1], 0.5,
                                        float(QW), Alu.mult, Alu.add)
                nc.vector.tensor_reduce(ch[:, 4:5], ch[:, 2:4], AxX, Alu.add)
                nc.vector.tensor_tensor(cl[:, j:j + 1], cl[:, j:j + 1],
                                        ch[:, 4:5], Alu.add)

        def interp2_th(pool, cl, lmids, th_out, g, name):
            """log-log linear interp of count->kk over 2 rungs."""
            lc = pool.tile([128, 2], f32, name=f"{name}_lc{g}")
            nc.vector.tensor_scalar(lc[:], cl[:], 1.0, None, Alu.max)
            nc.scalar.activation(lc[:], lc[:], Act.Ln)
            num = pool.tile([128, 1], f32, name=f"{name}_num{g}")
            den = pool.tile([128, 1], f32, name=f"{name}_den{g}")
            dl = pool.tile([128, 1], f32, name=f"{name}_dl{g}")
            nc.vector.tensor_scalar(num[:], lc[:, 0:1], lkk_b[:], None,
                                    Alu.subtract)
            nc.vector.tensor_scalar(den[:], lc[:, 0:1], lc[:, 1:2], None,
                                    Alu.subtract)
            nc.vector.tensor_scalar(den[:], den[:], 1e-5, None, Alu.max)
            nc.vector.tensor_scalar(dl[:], lmids[:, 1:2], lmids[:, 0:1], None,
                                    Alu.subtract)
            nc.vector.reciprocal(den[:], den[:])
            nc.vector.tensor_tensor(num[:], num[:], den[:], Alu.mult)
            nc.vector.tensor_tensor(num[:], num[:], dl[:], Alu.mult)
            nc.vector.tensor_scalar(num[:], num[:], lmids[:, 0:1], None, Alu.add)
            nc.scalar.activation(th_out[:], num[:], Act.Exp)

        def interp_th(pool, cl, lmids, th_out, g, name):
            """log-log piecewise-linear interp of count->kk over 3 rungs."""
            lc = pool.tile([128, 3], f32, name=f"{name}_lc{g}")
            nc.vector.tensor_scalar(lc[:], cl[:], 1.0, None, Alu.max)
            nc.scalar.activation(lc[:], lc[:], Act.Ln)
            shi = pool.tile([128, 1], f32, name=f"{name}_shi{g}")
            nc.vector.tensor_scalar(shi[:], cl[:, 1:2], kk_b[:], None, Alu.is_ge)
            slo = pool.tile([128, 1], f32, name=f"{name}_slo{g}")
            nc.vector.tensor_scalar(slo[:], shi[:], -1.0, 1.0, Alu.mult, Alu.add)

            def blend(dst, a_hi, a_lo, tmp):
                nc.vector.tensor_tensor(dst, a_hi, shi[:], Alu.mult)
                nc.vector.tensor_tensor(tmp, a_lo, slo[:], Alu.mult)
                nc.vector.tensor_tensor(dst, dst, tmp, Alu.add)

            tmp = pool.tile([128, 1], f32, name=f"{name}_tmp{g}")
            num = pool.tile([128, 1], f32, name=f"{name}_num{g}")
            den = pool.tile([128, 1], f32, name=f"{name}_den{g}")
            base = pool.tile([128, 1], f32, name=f"{name}_base{g}")
            dl = pool.tile([128, 1], f32, name=f"{name}_dl{g}")
            d01 = pool.tile([128, 1], f32, name=f"{name}_d01{g}")
            d12 = pool.tile([128, 1], f32, name=f"{name}_d12{g}")
            # num = (lc[seg_lo_idx] - lkk)
            nc.vector.tensor_scalar(d01[:], lc[:, 1:2], lkk_b[:], None, Alu.subtract)
            nc.vector.tensor_scalar(d12[:], lc[:, 0:1], lkk_b[:], None, Alu.subtract)
            blend(num[:], d01[:], d12[:], tmp[:])
            # den = (lc[lo] - lc[hi])
            nc.vector.tensor_scalar(d01[:], lc[:, 1:2], lc[:, 2:3], None, Alu.subtract)
            nc.vector.tensor_scalar(d12[:], lc[:, 0:1], lc[:, 1:2], None, Alu.subtract)
            blend(den[:], d01[:], d12[:], tmp[:])
            nc.vector.tensor_scalar(den[:], den[:], 1e-5, None, Alu.max)
            # base / dl
            blend(base[:], lmids[:, 1:2], lmids[:, 0:1], tmp[:])
            nc.vector.tensor_scalar(d01[:], lmids[:, 2:3], lmids[:, 1:2], None,
                                    Alu.subtract)
            nc.vector.tensor_scalar(d12[:], lmids[:, 1:2], lmids[:, 0:1], None,
                                    Alu.subtract)
            blend(dl[:], d01[:], d12[:], tmp[:])
            nc.vector.reciprocal(den[:], den[:])
            nc.vector.tensor_tensor(num[:], num[:], den[:], Alu.mult)
            nc.vector.tensor_tensor(num[:], num[:], dl[:], Alu.mult)
            nc.vector.tensor_tensor(base[:], base[:], num[:], Alu.add)
            nc.scalar.activation(th_out[:], base[:], Act.Exp)

        # =============== P1: |F| -> fp16 + two-stage ladder ===============
        t_stage = pool_dram.tile([S, NL1], f32, name="t_stage")
        t_out = pool_dram.tile([S, NL1], f32, name="t_out", addr_space="Shared")
        t2_stage = pool_dram.tile([S, NL2], f32, name="t2_stage")
        t2_out = pool_dram.tile([S, NL2], f32, name="t2_out", addr_space="Shared")

        e1 = [float(np.exp(-DLT1)), float(np.exp(DLT1))]
        e2 = [float(np.exp(-DLT2)), 1.0, float(np.exp(DLT2))]

        with tc.tile_pool(name="selpool", bufs=1) as sp:
            A16 = sp.tile([128, NG * FREE], f16, name="A16")
            scr_v = sp.tile([128, QW], f16, name="scr_v")
            scr_s = sp.tile([128, QW], f16, name="scr_s")

            for g in range(NG):
                def consume_p1(c, ps, g=g):
                    nc.scalar.activation(
                        A16[:, g * FREE + c * 512: g * FREE + (c + 1) * 512],
                        ps[:], Act.Abs, scale=inten[g][:])
                flow_pass_r(g, consume_p1, sp)

            # stage 1
            lm1 = []
            for g in range(NG):
                mids, lmids, nmids = build_rungs(sp, t0[g], e1, g, "s1")
                lm1.append(lmids)
                cl = sp.tile([128, NL1], f32, name=f"cl1_{g}")
                count_rungs(sp, A16[:, g * FREE:(g + 1) * FREE], mids, nmids,
                            NL1, cl, scr_v, scr_s, g, "s1")
                dma(t_stage[g * 128:(g + 1) * 128, :], cl[:])
                if DEBUG:
                    dma(dbg["dbg_mid"][g * 128:(g + 1) * 128, 0:NL1], mids[:])
            nc.gpsimd.collective_compute(
                "AllReduce", Alu.add, replica_groups=RG,
                ins=[t_stage[:]], outs=[t_out[:]])
            for g in range(NG):
                cl = sp.tile([128, NL1], f32, name=f"cl1g_{g}")
                dma(cl[:], t_out[g * 128:(g + 1) * 128, :])
                if DEBUG:
                    dma(dbg["dbg_cnt"][g * 128:(g + 1) * 128, 0:NL1], cl[:])
                interp2_th(sp, cl, lm1[g], th1[g][:], g, "i1")

            # stage 2
            lm2 = []
            for g in range(NG):
                mids, lmids, nmids = build_rungs(sp, th1[g], e2, g, "s2")
                lm2.append(lmids)
                cl = sp.tile([128, NL2], f32, name=f"cl2_{g}")
                count_rungs(sp, A16[:, g * FREE:(g + 1) * FREE], mids, nmids,
                            NL2, cl, scr_v, scr_s, g, "s2")
                dma(t2_stage[g * 128:(g + 1) * 128, :], cl[:])
            nc.gpsimd.collective_compute(
                "AllReduce", Alu.add, replica_groups=RG,
                ins=[t2_stage[:]], outs=[t2_out[:]])
            for g in range(NG):
                cl = sp.tile([128, NL2], f32, name=f"cl2g_{g}")
                dma(cl[:], t2_out[g * 128:(g + 1) * 128, :])
                if DEBUG:
                    dma(dbg["dbg_cnt2"][g * 128:(g + 1) * 128, 0:NL2], cl[:])
                interp_th(sp, cl, lm2[g], th2[g][:], g, "i2")
                if DEBUG:
                    dma(dbg["dbg_th"][g * 128:(g + 1) * 128, 0:1], th1[g][:])
                    dma(dbg["dbg_th"][g * 128:(g + 1) * 128, 1:2], th2[g][:])

        # =============== P4: final masked matvec (fp32 pass) ===============
        fo_stage = pool_dram.tile([S, ISLICE], f32, name="fo_stage")
        fo_out = pool_dram.tile([NCORES, S, ISLICE], f32, name="fo_out",
                                addr_space="Shared")
        tailP = ctx.enter_context(tc.tile_pool(name="tailP", bufs=1))
        fo_full = [tailP.tile([128, D], f32, name=f"fo_full{g}") for g in range(NG)]
        with tc.tile_pool(name="p4pool", bufs=1) as fp:
            XI16 = []
            for g in range(NG):
                t = fp.tile([128, D], f16, name=f"XI16_{g}")
                nc.vector.tensor_scalar(t[:], xn[g][:], inten[g][:], None, Alu.mult)
                XI16.append(t)
            for g in range(NG):
                FO = fp.tile([128, ISLICE], f32, name=f"FO{g}")

                def consume_p4(c, ps, g=g, FO=FO):
                    At = fp.tile([128, 512], f32, name="At", tag="At", bufs=3)
                    FM = fp.tile([128, 512], f16, name="FM", tag="FM", bufs=3)
                    sc16 = fp.tile([128, 512], f16, name="sc16", tag="sc16", bufs=3)
                    nc.scalar.activation(At[:], ps[:], Act.Abs, scale=inten[g][:])
                    nc.vector.scalar_tensor_tensor(FM[:], At[:], th2[g][:], ps[:],
                                                   Alu.is_ge, Alu.mult)
                    nc.vector.scalar_tensor_tensor(sc16[:], FM[:], 1.0, XI16[g][:],
                                                   Alu.mult, Alu.mult,
                                                   accum_out=FO[:, c:c + 1])
                flow_pass_hl(g, consume_p4, fp)
                dma(fo_stage[g * 128:(g + 1) * 128, :], FO[:])

        nc.gpsimd.collective_compute(
            "AllGather", Alu.bypass, replica_groups=RG,
            ins=[fo_stage[:]], outs=[fo_out[:]])

        # =============== tail ===============
        co = [tailP.tile([128, D], f32, name=f"co{g}") for g in range(NG)]
        with tc.tile_pool(name="tail1", bufs=1) as tp:
            n2g_b = bcast_row(tp, n2_g, D, "n2g_b")
            n2b_b = bcast_row(tp, n2_b, D, "n2b_b")
            for g in range(NG):
                for cidx in range(NCORES):
                    dma(fo_full[g][:, cidx * ISLICE:(cidx + 1) * ISLICE],
                        fo_out[cidx, g * 128:(g + 1) * 128, :])
                if DEBUG:
                    dma(dbg["dbg_fo"][g * 128:(g + 1) * 128, :], fo_full[g][:])
                nc.vector.tensor_tensor(co[g][:], xg[g][:], fo_full[g][:], Alu.add)
                mean = tp.tile([128, 1], f32, name=f"mean2{g}")
                m2 = tp.tile([128, 1], f32, name=f"m2ln2{g}")
                tmp = tp.tile([128, D], f32, name=f"ln2tmp{g}", tag="tmp")
                nc.vector.tensor_reduce(mean[:], co[g][:], AxX, Alu.add)
                nc.vector.tensor_scalar(mean[:], mean[:], 1.0 / D, None, Alu.mult)
                nc.vector.tensor_scalar(tmp[:], co[g][:], mean[:], None,
                                        Alu.subtract)
                nc.vector.scalar_tensor_tensor(tmp[:], tmp[:], 1.0, tmp[:], Alu.mult,
                                               Alu.mult, accum_out=m2[:])
                nc.vector.tensor_scalar(m2[:], m2[:], 1.0 / D, 1e-5, Alu.mult,
                                        Alu.add)
                rstd = tp.tile([128, 1], f32, name=f"rstd2{g}")
                nc.scalar.activation(rstd[:], m2[:], Act.Sqrt)
                nc.vector.reciprocal(rstd[:], rstd[:])
                nc.vector.tensor_scalar(co[g][:], co[g][:], mean[:], rstd[:],
                                        Alu.subtract, Alu.mult)
                nc.vector.scalar_tensor_tensor(co[g][:], co[g][:], 1.0, n2g_b[:],
                                               Alu.mult, Alu.mult)
                nc.vector.tensor_tensor(co[g][:], co[g][:], n2b_b[:], Alu.add)

        def transposed_cols(pool, src_list, K, name):
            nk = K // 128
            tT = pool.tile([128, nk * S], f32r, name=f"{name}_T")
            for g in range(NG):
                for kc in range(nk):
                    transpose_to(tT[:, kc * S + g * 128: kc * S + (g + 1) * 128],
                                 src_list[g][:, kc * 128:(kc + 1) * 128],
                                 f"{name}T{g}_{kc}")
            return lambda g, kc: tT[:, kc * S + g * 128: kc * S + (g + 1) * 128]

        def big_matmul(pool, lhsT_cols, w_dram, K, N, name, bias_dram=None,
                       const_lhsT=None, out_list=None):
            nk = K // 128
            wsb = pool.tile([128, nk * N], f32r, name=f"{name}_wsb")
            for kc in range(nk):
                dma(wsb[:, kc * N:(kc + 1) * N], w_dram[kc * 128:(kc + 1) * 128, :])
            bias_b = (bcast_row(pool, bias_dram, N, f"{name}_bias")
                      if bias_dram is not None else None)
            cvec_b = None
            if const_lhsT is not None:
                cps = pool_ps.tile([1, N], f32, name="cps", tag="Tps",
                                   padded_shape=[128, 512])
                for kc in range(nk):
                    nc.tensor.matmul(cps[:1, :], const_lhsT[:, kc:kc + 1],
                                     wsb[:, kc * N:(kc + 1) * N],
                                     start=(kc == 0), stop=(kc == nk - 1))
                cvec = pool.tile([1, N], f32, name=f"{name}_cvec")
                nc.vector.tensor_copy(cvec[:], cps[:1, :])
                cvec_b = pool.tile([128, N], f32, name=f"{name}_cvecb")
                pbcast(pool, cvec_b[:], cvec[:], N, f"{name}cv")
            outs = []
            for g in range(NG):
                o = (out_list[g] if out_list is not None
                     else pool.tile([128, N], f32, name=f"{name}_o{g}"))
                for nb in range(0, N, 512):
                    nw = min(512, N - nb)
                    ps = pool_mm.tile([128, nw], f32, name="Fps", tag="Fps")
                    for kc in range(nk):
                        nc.tensor.matmul(ps[:], lhsT_cols(g, kc),
                                         wsb[:, kc * N + nb: kc * N + nb + nw],
                                         start=(kc == 0), stop=(kc == nk - 1))
                    nc.vector.tensor_copy(o[:, nb:nb + nw], ps[:])
                if bias_b is not None:
                    nc.vector.tensor_tensor(o[:], o[:], bias_b[:], Alu.add)
                if cvec_b is not None:
                    nc.vector.tensor_tensor(o[:], o[:], cvec_b[:], Alu.add)
                outs.append(o)
            return outs

        # memory-bank mean -> memvT [D,1] as 4 chunks
        with tc.tile_pool(name="tailmem", bufs=1) as mp:
            memx = mp.tile([128, 4 * D], f32, name="memx")
            for kc in range(4):
                dma(memx[:, kc * D:(kc + 1) * D],
                    memory_bank[kc * 128:(kc + 1) * 128, :])
            mem_ps = pool_ps.tile([1, D], f32, name="memps", tag="Tps",
                                  padded_shape=[128, 512])
            for kc in range(4):
                nc.tensor.matmul(mem_ps[:1, :], ones_sb[:],
                                 memx[:, kc * D:(kc + 1) * D],
                                 start=(kc == 0), stop=(kc == 3))
            memv = mp.tile([1, D], f32, name="memv")
            nc.vector.tensor_scalar(memv[:], mem_ps[:1, :], 1.0 / 512.0, None,
                                    Alu.mult)
            memvT = tailP.tile([128, 4], f32r, name="memvT")
            for kc in range(4):
                transpose_to(memvT[:, kc:kc + 1], memv[:, kc * 128:(kc + 1) * 128],
                             f"memvT{kc}")

        with tc.tile_pool(name="tailA", bufs=1) as ta_:
            coT = transposed_cols(ta_, co, D, "coT")
            mh = big_matmul(ta_, coT, mem_w1, D, D, "memh", bias_dram=mem_b1,
                            const_lhsT=memvT)
            for g in range(NG):
                silu_(ta_, mh[g][:], mh[g][:], f"mh{g}")
            mhT = transposed_cols(ta_, mh, D, "mhT")
            mo = big_matmul(ta_, mhT, mem_w2, D, D, "memo", bias_dram=mem_b2)
            for g in range(NG):
                nc.vector.tensor_tensor(co[g][:], co[g][:], mo[g][:], Alu.add)

        gv = [tailP.tile([128, 4 * D], f32, name=f"gv{g}") for g in range(NG)]
        with tc.tile_pool(name="tailB", bufs=1) as tb_:
            coT2 = transposed_cols(tb_, co, D, "coT2")
            ff = big_matmul(tb_, coT2, up_w, D, 8 * D, "ff", bias_dram=up_b)
            for g in range(NG):
                silu_(tb_, gv[g][:], ff[g][:, :4 * D], f"gv{g}")
                nc.vector.tensor_tensor(gv[g][:], gv[g][:], ff[g][:, 4 * D:],
                                        Alu.mult)
        with tc.tile_pool(name="tailC", bufs=1) as tcp:
            gvT = transposed_cols(tcp, gv, 4 * D, "gvT")
            ffn = big_matmul(tcp, gvT, down_w, 4 * D, D, "ffn", bias_dram=down_b)
            for g in range(NG):
                nc.vector.tensor_tensor(ffn[g][:], ffn[g][:], co[g][:], Alu.add)
                dma(out_dram[g * 128:(g + 1) * 128, :], ffn[g][:])

    return nc


def _install_ntff_shim():
    """Reconstitute the missing antenv.axon_hooks module so
    run_bass_kernel_spmd(trace=True) can reach the axon NTFF profiler."""
    import sys
    import types

    if "antenv.axon_hooks" in sys.modules:
        return
    import antenv

    mod = types.ModuleType("antenv.axon_hooks")
    _h = [None]
    mod.set_axon_ntff_profile_hook = lambda h: _h.__setitem__(0, h)
    mod.get_axon_ntff_profile_hook = lambda: _h[0]
    sys.modules["antenv.axon_hooks"] = mod
    antenv.axon_hooks = mod
    try:
        from trn_agent_boot.trn_boot import _ntff_profile_via_ctypes

        mod.set_axon_ntff_profile_hook(
            _ntff_profile_via_ctypes("/opt/axon/libaxon_pjrt.so"))
    except Exception:
        pass


def kernel(**inputs):
    from concourse.bass_utils import run_bass_kernel_spmd
    _install_ntff_shim()

    sin, cos, qpoly = _host_constants()
    x = np.ascontiguousarray(np.asarray(inputs["x"], np.float32).reshape(S, D))
    patterns = np.ascontiguousarray(np.asarray(inputs["flow_patterns"], np.float32))

    nc = build_kernel()
    nc.finalize()

    def a(k):
        return np.ascontiguousarray(np.asarray(inputs[k], np.float32))

    def row(k):
        return np.ascontiguousarray(np.asarray(inputs[k], np.float32).reshape(1, -1))

    base = {
        "x": x,
        "sel_w1": a("sel_w1"), "sel_b1": row("sel_b1"),
        "sel_w2": a("sel_w2"), "sel_b2": row("sel_b2"),
        "win_w1": a("win_w1"), "win_b1": row("win_b1"),
        "win_w2": a("win_w2"), "win_b2": row("win_b2"),
        "int_w1": a("int_w1"), "int_b1": row("int_b1"),
        "int_w2": a("int_w2"), "int_b2": row("int_b2"),
        "mem_w1": a("mem_w1"), "mem_b1": row("mem_b1"),
        "mem_w2": a("mem_w2"), "mem_b2": row("mem_b2"),
        "memory_bank": a("memory_bank"),
        "up_w": a("up_w"), "up_b": row("up_b"),
        "down_w": a("down_w"), "down_b": row("down_b"),
        "n1_g": row("n1_g"), "n1_b": row("n1_b"),
        "n2_g": row("n2_g"), "n2_b": row("n2_b"),
        "rope_sin": sin, "rope_cos": cos,
        "qpoly": qpoly.reshape(1, 4),
    }
    import ml_dtypes
    in_maps = []
    for c in range(NCORES):
        m = dict(base)
        psl = np.ascontiguousarray(
            patterns[:, c * ISLICE:(c + 1) * ISLICE, :].reshape(P, FREE))
        m["pat_r"] = psl
        phi = psl.astype(ml_dtypes.bfloat16)
        m["pat_hi"] = phi
        m["pat_lo"] = (psl - phi.astype(np.float32)).astype(ml_dtypes.bfloat16)
        # [FREE, P] -> [128, (FREE/128)*P]: partition p holds rows p, p+128, ...
        m["pat_T"] = np.ascontiguousarray(
            psl.T.reshape(FREE // 128, 128, P).transpose(1, 0, 2).reshape(
                128, (FREE // 128) * P))
        in_maps.append(m)

    trace = os.environ.get("KERNEL_TRACE", "0") == "1"
    res = run_bass_kernel_spmd(nc, in_maps, list(range(NCORES)), trace=trace)
    out0 = res.results[0]
    kernel.last_results = res.results
    kernel.last_exec_ns = getattr(res, "exec_time_ns", None)
    return out0["out"].reshape(B, S, D).astype(np.float32)


if __name__ == "__main__":
    data = np.load("/tmp/inputs.npz")
    inputs = {k: data[k] for k in data.files}
    out = kernel(**inputs)
    print("out", out.shape, float(np.abs(out).max()))
